# revision 38
# baseline (speedup 1.0000x reference)
"""Trainium2 Bass kernel for nn_Attention (B=4, S=1024, DIM=1024, H=16, Dh=64).

Sharding: 8 cores = 4 batches x 2 head-groups (8 heads / 512 inner channels
each).  Each core computes q/k/v projections for its head shard, RoPE,
attention, and a partial output projection (its rows of Wo); the host sums
the two head-group partials per batch, adds the output bias and applies the
query mask (the cheap elementwise epilogue of the unshard step).

Device dataflow (per core), matmul operands in fp16 (fp32 PSUM accumulate):
  inputs stream in per-kt chunks on three HW DMA queues; the first 7
  projection chains (K/Q row-tile 0 + V st 0-2) interleave per-kt so the PE
  starts ~1us in and never waits on HBM.
  Q^T,K^T = W^T @ x^T  (bias added on the PSUM->SBUF pass); RoPE on the
  first 64 flat channels (host sends cos=1/sin=0 elsewhere).
  Phase 2 runs q-half (c2) OUTER, head-pair row-tile (mt) inner:
    scores^T[k,q] = K_h @ Q_h^T      (row-tiled pair, concurrent)
    P^T = exp(scores^T/8 + maskb[k]) (one ACT op per (mt,kt) over 2h x 512q)
    attn^T += V^T @ P^T              (col-tiled pair, concurrent)
    rowsums += ones @ P^T            (col-tiled M=1 pair, concurrent)
  as one flat 64-group software pipeline (PV/rowsum lag the exp by one
  group, crossing mt/c2 boundaries) with the mt1-3 K/Q projection matmuls
  zippered into the q-half-0 groups and the q-half-0 output-projection
  chains zippered into the q-half-1 groups, so the PE stays dense while
  ACT streams exps.  NOTE: zipper chains must be fully EMITTED before the
  lookahead scores emission that reads their output (emission order
  defines Tile's dependencies).
  Normalization: approx-reciprocal of rowsums; DRAM-bounce partition
  broadcast + gpsimd multiply for q-half 0 (long slack), PE ones-matmul
  broadcast + DVE multiply for q-half 1 (SWDGE bounce latency would sit
  on the tail).
  out[q,:] = attn_norm^T.T @ Wo_shard, written as fp16; host adds bo and
  masks.
"""

import numpy as np

B, S, DIM, HEADS, HEAD_DIM = 4, 1024, 1024, 16, 64
INNER = HEADS * HEAD_DIM
HG = 2                      # head groups (tensor-parallel shards)
DSH = INNER // HG           # 512 inner channels per core
HSH = HEADS // HG           # 8 heads per core
NCORES = B * HG
KT = DIM // 128             # 8 contraction tiles
MT = DSH // 128             # 4 row tiles (head pairs)
ST = S // 128               # 8 seq tiles
MASK_NEG = -80.0

_CACHE = {}


def _build():
    import concourse.tile as tile
    from concourse import bacc, mybir

    f32 = mybir.dt.float32
    f16 = mybir.dt.float16
    f8 = mybir.dt.float8e4
    AF = mybir.ActivationFunctionType
    OP = mybir.AluOpType
    DR = mybir.MatmulPerfMode.DoubleRow

    nc = bacc.Bacc("TRN2", target_bir_lowering=False, debug=False)

    xT_d = nc.dram_tensor("xT", [128, KT, S], f16, kind="ExternalInput")
    xT8_d = nc.dram_tensor("xT8", [128, KT, S], f8, kind="ExternalInput")
    wq_d = nc.dram_tensor("wq", [128, MT, KT, 128], f16, kind="ExternalInput")
    wk_d = nc.dram_tensor("wk", [128, MT, KT, 128], f8, kind="ExternalInput")
    wv_d = nc.dram_tensor("wv", [128, KT, DSH], f16, kind="ExternalInput")
    wo_d = nc.dram_tensor("wo", [128, MT, DIM], f16, kind="ExternalInput")
    bq_d = nc.dram_tensor("bq", [128, MT], f32, kind="ExternalInput")
    bk_d = nc.dram_tensor("bk", [128, MT], f32, kind="ExternalInput")
    bv_d = nc.dram_tensor("bv", [1, DSH], f16, kind="ExternalInput")
    cos_d = nc.dram_tensor("cos2", [128, S], f32, kind="ExternalInput")
    sin_d = nc.dram_tensor("sin2", [128, S], f32, kind="ExternalInput")
    prt_d = nc.dram_tensor("prt", [128, 128], f16, kind="ExternalInput")
    maskb_d = nc.dram_tensor("maskb", [128, ST], f32, kind="ExternalInput")
    out_d = nc.dram_tensor("out", [S, DIM], f16, kind="ExternalOutput")
    recd = nc.dram_tensor("recd", [97, MT, 512], f16)

    with tile.TileContext(nc) as tc, \
         tc.tile_pool(name="sb", bufs=1) as sb, \
         tc.tile_pool(name="psp", bufs=1, space="PSUM") as psp:

        # ---- persistent SBUF ------------------------------------------
        xT = sb.tile([128, KT, S], f16)
        xT8 = sb.tile([128, KT, S], f8)
        wq = sb.tile([128, MT, KT, 128], f16)
        wk = sb.tile([128, MT, KT, 128], f8)
        wv = sb.tile([128, KT, DSH], f16)
        wo = sb.tile([128, MT, DIM], f16)
        bq = sb.tile([128, MT], f32)
        bk = sb.tile([128, MT], f32)
        bvb = sb.tile([128, HSH, HEAD_DIM], f16)
        cos2 = sb.tile([128, S], f32)
        sin2 = sb.tile([128, S], f32)
        prt = sb.tile([128, 128], f16)
        maskb = sb.tile([128, ST], f32)
        qT = sb.tile([128, MT, S], f16)
        kT = sb.tile([128, MT, S], f16)
        # V with a ones column appended per head: the PV matmul (M=65)
        # then produces the softmax rowsum at output row 64 for free,
        # replacing the 128 M=1 rowsum matmuls (~20us of PE streaming).
        vv = sb.tile([128, ST, HSH, HEAD_DIM + 1], f16)
        attU = sb.tile([128, MT, S], f16)
        # rowsum staging at partition 0: reciprocal_approx_fast only works
        # with base-partition-0 operands (HW-probed; base 64 reads garbage)
        rss2 = sb.tile([1, 2, 512], f32)
        recf2 = sb.tile([1, 2, 512], f32)
        recq = sb.tile([97, MT, 512], f16)
        ones64 = sb.tile([97, HEAD_DIM], f16)

        # ---- input DMAs: per-kt chunks on three HW DMA queues ---------
        # sync: xT f16 halves, then wo; scalar: wk8/wq8 (mt0 first) + xT8
        # + smalls + rest; gpsimd: wv.
        nc.gpsimd.dma_start(
            out=bvb[:].rearrange("p h d -> p (h d)"),
            in_=bv_d.ap()[0:1, :].partition_broadcast(128))
        for kt in range(KT):
            nc.sync.dma_start(out=xT[:, kt, 0:512],
                              in_=xT_d.ap()[:, kt, 0:512])
            nc.sync.dma_start(out=xT[:, kt, 512:1024],
                              in_=xT_d.ap()[:, kt, 512:1024])
            nc.gpsimd.dma_start(out=wv[:, kt], in_=wv_d.ap()[:, kt])
            nc.scalar.dma_start(out=wk[:, 0, kt], in_=wk_d.ap()[:, 0, kt])
            nc.scalar.dma_start(out=wq[:, 0, kt], in_=wq_d.ap()[:, 0, kt])
            nc.scalar.dma_start(out=xT8[:, kt], in_=xT8_d.ap()[:, kt])
            if kt == 1:
                for t, d in [(bq, bq_d), (bk, bk_d), (maskb, maskb_d)]:
                    nc.scalar.dma_start(out=t[:], in_=d.ap())
        nc.scalar.dma_start(out=prt[:], in_=prt_d.ap())
        nc.scalar.dma_start(out=cos2[:], in_=cos_d.ap())
        nc.scalar.dma_start(out=sin2[:], in_=sin_d.ap())
        for mt in range(1, MT):
            nc.scalar.dma_start(out=wk[:, mt], in_=wk_d.ap()[:, mt])
            nc.scalar.dma_start(out=wq[:, mt], in_=wq_d.ap()[:, mt])
        for mt in range(MT):
            nc.gpsimd.dma_start(out=wo[:, mt], in_=wo_d.ap()[:, mt])

        ones_f = sb.tile([128, S], f32)
        nc.vector.memset(ones_f[:], 1.0)
        nc.vector.memset(
            vv[:, :, :, HEAD_DIM:HEAD_DIM + 1]
            .rearrange("p s h one -> p (s h one)"), 1.0)
        # only rows {0,32,64,96} of recq get real rowsums, but the bounce
        # DMA ships all 97 rows — initialize so the unused rows are defined
        nc.vector.memset(recq[:], 0.0)
        nc.vector.tensor_copy(ones64[:], ones_f[0:97, 0:HEAD_DIM])
        # tiny dummy exp: pulls the ~2.7us ACT table load into the DMA fill
        # and keeps the tile scheduler's model of the first real exps tight
        warm = sb.tile([1, 8], f32)
        with tc.high_priority():
            nc.scalar.activation(warm[:], ones_f[0:1, 0:8], AF.Exp)

        def rope_apply(dst, b, c2, ps, sbpool):
            # row-tile 0 only: RoPE on the first 64 flat channels (rows
            # 64-127 and the hg=1 core get identity via cos=1/sin=0).
            sl = slice(c2 * 512, (c2 + 1) * 512)
            sinp = sbpool.tile([128, 512], f16, tag="sinp", name="sinp")
            nc.vector.scalar_tensor_tensor(
                sinp[:], ps[:], b[:, 0:1], sin2[:, sl],
                op0=OP.add, op1=OP.mult)
            cosp = sbpool.tile([128, 512], f32, tag="cosp", name="cosp")
            nc.vector.scalar_tensor_tensor(
                cosp[:], ps[:], b[:, 0:1], cos2[:, sl],
                op0=OP.add, op1=OP.mult)
            pp = psp.tile([128, 512], f32, tag="ps", name="pp")
            nc.tensor.matmul(out=pp[:], lhsT=prt[:], rhs=sinp[:],
                             start=True, stop=True)
            nc.vector.tensor_tensor(dst[:, 0, sl], cosp[:], pp[:], op=OP.add)

        # ---- prologue: mt0 K/Q chains + V, interleaved per kt ---------
        # so compute paces with the arriving per-kt DMA chunks.
        with tc.tile_pool(name="pkq", bufs=4, space="PSUM") as pkq, \
             tc.tile_pool(name="pvv", bufs=3, space="PSUM") as pvv, \
             tc.tile_pool(name="psb", bufs=2) as psb:

            def v_spill(st, ps):
                # bias add fused into the PSUM->SBUF spill (writes only the
                # 64 real V channels; col 64 stays the memset ones)
                nc.vector.tensor_tensor(
                    vv[:, st, :, 0:HEAD_DIM],
                    ps[:].rearrange("p (h d) -> p h d", h=HSH),
                    bvb[:], op=OP.add)

            kq = []
            for dst, w, b, dr in ((kT, wk, bk, True), (qT, wq, bq, False)):
                for c2 in range(2):
                    kq.append((dst, w, b, c2, dr,
                               pkq.tile([128, 512], f32, tag="kq",
                                        name="kq")))
            vps = [pvv.tile([128, DSH], f32, tag="vps", name="vps")
                   for _ in range(3)]
            for kt in range(KT):
                for dst, w, b, c2, dr, ps in kq:
                    sl = slice(c2 * 512, (c2 + 1) * 512)
                    if dr and kt % 2 == 1:
                        # K-side fp8 DoubleRow: one MM per kt-pair
                        nc.tensor.matmul(
                            out=ps[:], lhsT=w[:, 0, kt - 1:kt + 1, :],
                            rhs=xT8[:, kt - 1:kt + 1, sl],
                            start=(kt == 1), stop=(kt == KT - 1),
                            perf_mode=DR)
                    elif not dr:
                        nc.tensor.matmul(
                            out=ps[:], lhsT=w[:, 0, kt, :],
                            rhs=xT[:, kt, sl],
                            start=(kt == 0), stop=(kt == KT - 1))
                for st in range(3):
                    nc.tensor.matmul(
                        out=vps[st][:],
                        lhsT=xT[:, kt, st * 128:(st + 1) * 128],
                        rhs=wv[:, kt, :],
                        start=(kt == 0), stop=(kt == KT - 1))
            for st in range(3):
                v_spill(st, vps[st])

            # interleave the rope chains (DVE-heavy) and the K-mt1 chains
            # with the remaining V chains so neither engine idles in the
            # DMA-paced prologue tail
            def v_chain(st):
                ps = pvv.tile([128, DSH], f32, tag="vps", name="vps")
                for kt in range(KT):
                    nc.tensor.matmul(
                        out=ps[:], lhsT=xT[:, kt, st * 128:(st + 1) * 128],
                        rhs=wv[:, kt, :],
                        start=(kt == 0), stop=(kt == KT - 1))
                v_spill(st, ps)

            def k1_chain(half):
                ps = pvv.tile([128, DSH], f32, tag="vps", name="kx")
                sl = slice(half * 512, (half + 1) * 512)
                for k2 in range(KT // 2):
                    nc.tensor.matmul(
                        out=ps[:], lhsT=wk[:, 1, 2 * k2:2 * k2 + 2, :],
                        rhs=xT8[:, 2 * k2:2 * k2 + 2, sl],
                        start=(k2 == 0), stop=(k2 == KT // 2 - 1),
                        perf_mode=DR)
                nc.vector.tensor_scalar(
                    kT[:, 1, sl], ps[:], bk[:, 1:2], None, op0=OP.add)

            for st in range(3, ST):
                if st - 3 < len(kq):
                    dst, w, b, c2, dr, ps = kq[st - 3]
                    rope_apply(dst, b, c2, ps, psb)
                v_chain(st)
                if st == 3:
                    k1_chain(0)
                if st == 4:
                    k1_chain(1)

        # ---- zipper chains (run inside phase-2 groups) ----------------
        def chain_proj(dst, w, b, mt, half, dr=False):
            sl = slice(half * 512, (half + 1) * 512)
            ps = psp.tile([128, 512], f32, tag="ps", name="ps")
            if dr:  # K-side fp8 DoubleRow: 4 MMs over kt-pairs
                for k2 in range(KT // 2):
                    nc.tensor.matmul(
                        out=ps[:], lhsT=w[:, mt, 2 * k2:2 * k2 + 2, :],
                        rhs=xT8[:, 2 * k2:2 * k2 + 2, sl],
                        start=(k2 == 0), stop=(k2 == KT // 2 - 1),
                        perf_mode=DR)
                    if k2 == KT // 2 - 1:
                        nc.vector.tensor_scalar(
                            dst[:, mt, sl], ps[:], b[:, mt:mt + 1],
                            None, op0=OP.add)
                    yield
            else:
                for kt in range(KT):
                    nc.tensor.matmul(
                        out=ps[:], lhsT=w[:, mt, kt, :], rhs=xT[:, kt, sl],
                        start=(kt == 0), stop=(kt == KT - 1))
                    if kt == KT - 1:
                        nc.vector.tensor_scalar(
                            dst[:, mt, sl], ps[:], b[:, mt:mt + 1],
                            None, op0=OP.add)
                    yield

        def chain_out(qt, n2, pool, tag, obpool, copy_eng, dma_eng=None):
            nsl = slice(n2 * 512, (n2 + 1) * 512)
            ps = pool.tile([128, 512], f32, tag=tag, name="ps3")
            for mt in range(MT):
                nc.tensor.matmul(
                    out=ps[:], lhsT=attU[:, mt, qt * 128:(qt + 1) * 128],
                    rhs=wo[:, mt, nsl],
                    start=(mt == 0), stop=(mt == MT - 1))
                if mt == MT - 1:
                    ob = obpool.tile([128, 512], f16, tag="ob", name="ob")
                    if copy_eng == "act":
                        nc.scalar.activation(ob[:], ps[:], AF.Copy)
                    else:
                        nc.vector.tensor_copy(ob[:], ps[:])
                    eng = dma_eng
                    if eng is None:
                        eng = nc.sync if (qt + n2) % 2 == 0 else nc.scalar
                    eng.dma_start(
                        out=out_d.ap()[qt * 128:(qt + 1) * 128, nsl],
                        in_=ob[:])
                yield

        # ---- phase 2+3: attention with zippered projections/output ----
        with tc.tile_pool(name="psc", bufs=2, space="PSUM") as psc, \
             tc.tile_pool(name="pat0", bufs=2, space="PSUM") as pat0, \
             tc.tile_pool(name="pat1", bufs=1, space="PSUM") as pat1, \
             tc.tile_pool(name="p2sb", bufs=3) as p2sb, \
             tc.tile_pool(name="p2r", bufs=2) as p2r:

            def emit_scores(mt, kt, c2):
                sch = psc.tile([128, S], f32, tag="sch", name="sch")
                qsl = slice(c2 * 512, (c2 + 1) * 512)
                for hh in range(2):
                    ph = hh * 64
                    nc.tensor.matmul(
                        out=sch[:, hh * 512:hh * 512 + 512],
                        lhsT=kT[ph:ph + 64, mt, kt * 128:(kt + 1) * 128],
                        rhs=qT[ph:ph + 64, mt, qsl],
                        start=True, stop=True, tile_position=(ph, 0))
                return sch

            def normalize_bounce(mt, c2):
                # rows 32*(2hh+c2) of recq hold 1/rowsum for head (2mt+hh),
                # q-half c2.  DRAM bounce partition-broadcasts each row; the
                # multiply runs on the otherwise-idle GpSimd engine.
                nc.gpsimd.dma_start(out=recd.ap()[:, mt, :],
                                    in_=recq[:, mt, :])
                qsl = slice(c2 * 512, (c2 + 1) * 512)
                for hh in range(2):
                    ph, r = hh * 64, 32 * (hh * 2 + c2)
                    rb = p2r.tile([128, 512], f32, tag="rb", name="rb")
                    nc.gpsimd.dma_start(
                        out=rb[ph:ph + 64],
                        in_=recd.ap()[r:r + 1, mt, :].partition_broadcast(64))
                    nc.gpsimd.tensor_tensor(
                        attU[ph:ph + 64, mt, qsl], attU[ph:ph + 64, mt, qsl],
                        rb[ph:ph + 64], op=OP.mult)

            def normalize_pe(mt, c2):
                # PE ones-matmul broadcast (for the final tile where the
                # DMA bounce latency would sit exposed)
                qsl = slice(c2 * 512, (c2 + 1) * 512)
                rbps = psp.tile([128, 512], f32, tag="ps", name="rbps")
                for hh in range(2):
                    ph, r = hh * 64, 32 * (hh * 2 + c2)
                    nc.tensor.matmul(
                        out=rbps[ph:ph + 64, :], lhsT=ones64[r:r + 1, :],
                        rhs=recq[r:r + 1, mt, :],
                        start=True, stop=True, tile_position=(r, ph))
                for hh in range(2):
                    ph = hh * 64
                    nc.vector.tensor_tensor(
                        attU[ph:ph + 64, mt, qsl], attU[ph:ph + 64, mt, qsl],
                        rbps[ph:ph + 64, :], op=OP.mult)

            def gen_c2_0():
                # order is deadline-driven: each chain must be fully
                # EMITTED before the lookahead scores emission that reads
                # it (program order defines Tile's deps); K-mt1 chains ran
                # in the prologue
                yield from chain_proj(qT, wq, bq, 1, 0)
                yield from chain_proj(qT, wq, bq, 2, 0)
                yield from chain_proj(kT, wk, bk, 2, 0, dr=True)
                yield from chain_proj(kT, wk, bk, 2, 1, dr=True)
                yield from chain_proj(qT, wq, bq, 3, 0)
                yield from chain_proj(kT, wk, bk, 3, 0, dr=True)
                yield from chain_proj(kT, wk, bk, 3, 1, dr=True)
                yield from chain_proj(qT, wq, bq, 1, 1)
                while True:
                    yield

            def gen_c2_1():
                yield from chain_proj(qT, wq, bq, 2, 1)
                yield from chain_proj(qT, wq, bq, 3, 1)
                for qt in range(4):
                    for n2 in range(2):
                        yield from chain_out(qt, n2, psp, "ps", p2sb, "dve")
                while True:
                    yield

            # ---- flat 64-group software pipeline --------------------------
            # group g = (c2, mt, kt); exp/scores run one group AHEAD of the
            # PV/rowsum consumers, crossing mt and c2 boundaries, so neither
            # the PE nor ACT ever drains at a boundary.
            ZIPN = {0: (2, 2, 1, 1), 1: (2, 2, 1, 1)}
            groups = [(c2, mt, kt) for c2 in range(2) for mt in range(MT)
                      for kt in range(ST)]
            gens = {0: gen_c2_0(), 1: gen_c2_1()}
            NG = len(groups)
            pts, ats = {}, {}
            sch = {0: emit_scores(0, 0, 0)}
            for g in range(NG + 1):
                if g < NG:
                    c2, mt, kt = groups[g]
                    pts[g] = p2sb.tile([128, S], f16, tag="pt", name="pt")
                    nc.scalar.activation(
                        pts[g][:], sch.pop(g)[:], AF.Exp,
                        bias=maskb[:, kt:kt + 1], scale=0.125)
                    if g + 1 < NG:
                        nc2, nmt, nkt = groups[g + 1]
                        sch[g + 1] = emit_scores(nmt, nkt, nc2)
                    for _ in range(ZIPN[c2][mt]):
                        next(gens[c2])
                if g >= 1:
                    c2, mt, kt = groups[g - 1]
                    first, last = (kt == 0), (kt == ST - 1)
                    if first:
                        ats[mt, c2] = (
                            pat0.tile([128, 512], f32, tag="at0", name="at0"),
                            pat1.tile([128, 512], f32, tag="at1", name="at1"))
                    atx = ats[mt, c2]
                    pt = pts.pop(g - 1)
                    for hh in range(2):  # PV pair, rowsum row merged (M=65)
                        nc.tensor.matmul(
                            out=atx[hh][0:65, :],
                            lhsT=vv[:, kt, mt * 2 + hh, :],
                            rhs=pt[:, hh * 512:hh * 512 + 512],
                            start=first, stop=last)
                    if last:
                        # epilogue: reciprocal chain first (it gates
                        # normalize), then the attn spill.  rowsums bounce
                        # through partition 0 (recip is base-0-only).
                        qsl = slice(c2 * 512, (c2 + 1) * 512)
                        for hh in range(2):
                            nc.vector.tensor_copy(
                                rss2[0:1, hh], atx[hh][64:65, :])
                        nc.vector.reciprocal_approx_fast(
                            recf2[0:1, :, :], rss2[0:1, :, :])
                        for hh in range(2):
                            r = 32 * (hh * 2 + c2)
                            nc.vector.tensor_copy(
                                recq[r:r + 1, mt, :], recf2[0:1, hh])
                            nc.vector.tensor_copy(
                                attU[hh * 64:hh * 64 + 64, mt, qsl],
                                atx[hh][0:64, :])
                        if c2 == 0 and mt < MT - 1:
                            normalize_bounce(mt, c2)
                        else:
                            normalize_pe(mt, c2)

            # ---- tail: output projection for q-half 1 -----------------
            pools = [(psp, "ps"), (pat0, "at0"), (pat1, "at1")]
            dma_engs = [nc.sync, nc.scalar, nc.gpsimd]
            for i, (qt, n2) in enumerate(
                    (q, n) for q in range(4, 8) for n in range(2)):
                pool, tag = pools[i % 3]
                eng = "act" if i % 2 == 0 else "dve"
                for _ in chain_out(qt, n2, pool, tag, p2sb, eng,
                                   dma_eng=dma_engs[i % 3]):
                    pass

    nc.compile()
    return nc


def _get_nc():
    if "nc" not in _CACHE:
        _CACHE["nc"] = _build()
    return _CACHE["nc"]


def _prep_inputs(x, mask, freqs, Wq, bq, Wk, bk, Wv, bv, Wo, bo):
    from ml_dtypes import float8_e4m3 as f8e4
    f = np.asarray(freqs, np.float32)[0]              # [S, HEAD_DIM]
    # reference rotates only the first rot_dim=64 channels of the FLAT
    # inner dim -> rows 0-63 of row-tile 0 on the hg=0 core; everything
    # else is identity (cos=1, sin=0).
    cos2 = np.ones((128, S), np.float32)
    sin2 = np.zeros((128, S), np.float32)
    cos2[0:HEAD_DIM] = np.cos(f.T)
    sin2[0:HEAD_DIM] = np.sin(f.T)
    ident = np.ones((128, S), np.float32)
    identz = np.zeros((128, S), np.float32)

    prt = np.zeros((128, 128), np.float16)            # P_rot^T
    i = np.arange(0, 128, 2)
    prt[i + 1, i] = -1.0                              # P_rot[2i, 2i+1] = -1
    prt[i, i + 1] = 1.0                               # P_rot[2i+1, 2i] = +1

    def lhsT_w(w, dt=np.float16):        # [DIM, DSH] -> [128, MT, KT, 128]
        return np.ascontiguousarray(
            w.reshape(KT, 128, MT, 128).transpose(1, 2, 0, 3)
        ).astype(dt)

    def col(b):                                       # [DSH] -> [128, MT]
        return np.ascontiguousarray(b.reshape(MT, 128).T.astype(np.float32))

    in_maps = []
    for b in range(B):
        xTf = np.ascontiguousarray(
            np.asarray(x[b], np.float32).T.reshape(KT, 128, S)
            .transpose(1, 0, 2))
        xT = xTf.astype(np.float16)
        xT8 = xTf.astype(f8e4)
        m = np.asarray(mask[b])
        maskb = np.ascontiguousarray(
            np.where(m, 0.0, MASK_NEG).astype(np.float32).reshape(ST, 128).T)
        for hg in range(HG):
            dsl = slice(hg * DSH, (hg + 1) * DSH)
            in_maps.append({
                "xT": xT,
                "xT8": xT8,
                "wq": lhsT_w(np.asarray(Wq, np.float32)[:, dsl]),
                "wk": lhsT_w(np.asarray(Wk, np.float32)[:, dsl], f8e4),
                "wv": np.ascontiguousarray(
                    np.asarray(Wv, np.float32)[:, dsl]
                    .reshape(KT, 128, DSH).transpose(1, 0, 2)).astype(np.float16),
                "wo": np.ascontiguousarray(
                    np.asarray(Wo, np.float32)[dsl, :]
                    .reshape(MT, 128, DIM).transpose(1, 0, 2)).astype(np.float16),
                "bq": col(np.asarray(bq, np.float32)[dsl]),
                "bk": col(np.asarray(bk, np.float32)[dsl]),
                "bv": np.asarray(bv, np.float32)[None, dsl]
                    .astype(np.float16).copy(),
                "cos2": cos2 if hg == 0 else ident,
                "sin2": sin2 if hg == 0 else identz,
                "prt": prt,
                "maskb": maskb,
            })
    return in_maps


def run(trace=False, **inputs):
    from concourse import bass_utils
    if trace:
        _install_ntff_hook()
    nc = _get_nc()
    in_maps = _prep_inputs(**inputs)
    res = bass_utils.run_bass_kernel_spmd(
        nc, in_maps, core_ids=list(range(NCORES)), trace=trace)
    mask = np.asarray(inputs["mask"])
    bo = np.asarray(inputs["bo"], np.float32)
    out = np.empty((B, S, DIM), np.float32)
    for b in range(B):
        p = (res.results[2 * b]["out"].astype(np.float32)
             + res.results[2 * b + 1]["out"].astype(np.float32) + bo)
        out[b] = np.where(mask[b][:, None], p, 0.0)
    return out, res


def kernel(**inputs):
    out, _ = run(trace=False, **inputs)
    return out


def _install_ntff_hook():
    """Register the axon NTFF profiling hook missing from the antenv stub."""
    import sys, types
    try:
        import antenv.axon_hooks  # noqa: F401
        return
    except ImportError:
        pass
    from trn_agent_boot.trn_boot import _ntff_profile_via_ctypes
    hook = _ntff_profile_via_ctypes('/opt/axon/libaxon_pjrt.so')
    mod = types.ModuleType('antenv.axon_hooks')
    mod.get_axon_ntff_profile_hook = lambda: hook
    mod.set_axon_ntff_profile_hook = lambda h: None
    sys.modules['antenv.axon_hooks'] = mod



# revision 50
# speedup vs baseline: 1.0312x; 1.0312x over previous
"""Trainium2 Bass kernel for nn_Attention (B=4, S=1024, DIM=1024, H=16, Dh=64).

Sharding: 8 cores = 4 batches x 2 head-groups (8 heads / 512 inner channels
each).  Each core computes q/k/v projections for its head shard, RoPE,
attention, and a partial output projection (its rows of Wo); the host sums
the two head-group partials per batch, adds the output bias and applies the
query mask (the cheap elementwise epilogue of the unshard step).

Device dataflow (per core), matmul operands in fp16 (fp32 PSUM accumulate):
  inputs stream in per-kt chunks on three HW DMA queues; the first 7
  projection chains (K/Q row-tile 0 + V st 0-2) interleave per-kt so the PE
  starts ~1us in and never waits on HBM.
  Q^T,K^T = W^T @ x^T  (bias added on the PSUM->SBUF pass); RoPE on the
  first 64 flat channels (host sends cos=1/sin=0 elsewhere).
  Phase 2 runs q-half (c2) OUTER, head-pair row-tile (mt) inner:
    scores^T[k,q] = K_h @ Q_h^T      (row-tiled pair, concurrent)
    P^T = exp(scores^T/8 + maskb[k]) (one ACT op per (mt,kt) over 2h x 512q)
    attn^T += V^T @ P^T              (col-tiled pair, concurrent)
    rowsums += ones @ P^T            (col-tiled M=1 pair, concurrent)
  as one flat 64-group software pipeline (PV/rowsum lag the exp by one
  group, crossing mt/c2 boundaries) with the mt1-3 K/Q projection matmuls
  zippered into the q-half-0 groups and the q-half-0 output-projection
  chains zippered into the q-half-1 groups, so the PE stays dense while
  ACT streams exps.  NOTE: zipper chains must be fully EMITTED before the
  lookahead scores emission that reads their output (emission order
  defines Tile's dependencies).
  Normalization: approx-reciprocal of rowsums; DRAM-bounce partition
  broadcast + gpsimd multiply for q-half 0 (long slack), PE ones-matmul
  broadcast + DVE multiply for q-half 1 (SWDGE bounce latency would sit
  on the tail).
  out[q,:] = attn_norm^T.T @ Wo_shard, written as fp16; host adds bo and
  masks.
"""

import numpy as np

B, S, DIM, HEADS, HEAD_DIM = 4, 1024, 1024, 16, 64
INNER = HEADS * HEAD_DIM
HG = 2                      # head groups (tensor-parallel shards)
DSH = INNER // HG           # 512 inner channels per core
HSH = HEADS // HG           # 8 heads per core
NCORES = B * HG
KT = DIM // 128             # 8 contraction tiles
MT = DSH // 128             # 4 row tiles (head pairs)
ST = S // 128               # 8 seq tiles
MASK_NEG = -80.0

_CACHE = {}


def _build():
    import concourse.tile as tile
    from concourse import bacc, mybir

    f32 = mybir.dt.float32
    f16 = mybir.dt.float16
    AF = mybir.ActivationFunctionType
    OP = mybir.AluOpType

    nc = bacc.Bacc("TRN2", target_bir_lowering=False, debug=False)

    xT_d = nc.dram_tensor("xT", [128, KT, S], f16, kind="ExternalInput")
    wq_d = nc.dram_tensor("wq", [128, MT, KT, 128], f16, kind="ExternalInput")
    wk_d = nc.dram_tensor("wk", [128, MT, KT, 128], f16, kind="ExternalInput")
    wv_d = nc.dram_tensor("wv", [128, KT, DSH], f16, kind="ExternalInput")
    wo_d = nc.dram_tensor("wo", [128, MT, DIM], f16, kind="ExternalInput")
    bq_d = nc.dram_tensor("bq", [128, MT], f32, kind="ExternalInput")
    bk_d = nc.dram_tensor("bk", [128, MT], f32, kind="ExternalInput")
    bv_d = nc.dram_tensor("bv", [1, DSH], f16, kind="ExternalInput")
    cos_d = nc.dram_tensor("cos2", [128, S], f32, kind="ExternalInput")
    sin_d = nc.dram_tensor("sin2", [128, S], f32, kind="ExternalInput")
    prt_d = nc.dram_tensor("prt", [128, 128], f16, kind="ExternalInput")
    maskb_d = nc.dram_tensor("maskb", [128, ST], f32, kind="ExternalInput")
    out_d = nc.dram_tensor("out", [S, DIM], f16, kind="ExternalOutput")
    recd = nc.dram_tensor("recd", [97, MT, 512], f16)

    with tile.TileContext(nc) as tc, \
         tc.tile_pool(name="sb", bufs=1) as sb, \
         tc.tile_pool(name="psp", bufs=1, space="PSUM") as psp:

        # ---- persistent SBUF ------------------------------------------
        xT = sb.tile([128, KT, S], f16)
        wq = sb.tile([128, MT, KT, 128], f16)
        wk = sb.tile([128, MT, KT, 128], f16)
        wv = sb.tile([128, KT, DSH], f16)
        wo = sb.tile([128, MT, DIM], f16)
        bq = sb.tile([128, MT], f32)
        bk = sb.tile([128, MT], f32)
        bvb = sb.tile([128, HSH, HEAD_DIM], f16)
        cos2 = sb.tile([128, S], f32)
        sin2 = sb.tile([128, S], f32)
        prt = sb.tile([128, 128], f16)
        maskb = sb.tile([128, ST], f32)
        qT = sb.tile([128, MT, S], f16)
        kT = sb.tile([128, MT, S], f16)
        # V with a ones column appended per head: the PV matmul (M=65)
        # then produces the softmax rowsum at output row 64 for free,
        # replacing the 128 M=1 rowsum matmuls (~20us of PE streaming).
        vv = sb.tile([128, ST, HSH, HEAD_DIM + 1], f16)
        attU = sb.tile([128, MT, S], f16)
        # rowsum staging at partition 0: reciprocal_approx_fast only works
        # with base-partition-0 operands (HW-probed; base 64 reads garbage)
        rss2 = sb.tile([1, 2, 512], f32)
        recf2 = sb.tile([1, 2, 512], f32)
        recq = sb.tile([97, MT, 512], f16)
        ones64 = sb.tile([97, HEAD_DIM], f16)

        # ---- input DMAs: per-kt chunks on three HW DMA queues ---------
        # sync: xT half-0 all kt + half-1 even kt; gpsimd: wv + xT half-1
        # odd kt, then wo; scalar: wk/wq (mt0 first) + smalls + mt1-3.
        nc.gpsimd.dma_start(
            out=bvb[:].rearrange("p h d -> p (h d)"),
            in_=bv_d.ap()[0:1, :].partition_broadcast(128))
        for kt in range(KT):
            nc.sync.dma_start(out=xT[:, kt, 0:512],
                              in_=xT_d.ap()[:, kt, 0:512])
            heng = nc.sync if kt % 2 == 0 else nc.gpsimd
            heng.dma_start(out=xT[:, kt, 512:1024],
                           in_=xT_d.ap()[:, kt, 512:1024])
            nc.gpsimd.dma_start(out=wv[:, kt], in_=wv_d.ap()[:, kt])
            nc.scalar.dma_start(out=wk[:, 0, kt], in_=wk_d.ap()[:, 0, kt])
            nc.scalar.dma_start(out=wq[:, 0, kt], in_=wq_d.ap()[:, 0, kt])
            if kt == 1:
                for t, d in [(bq, bq_d), (bk, bk_d), (maskb, maskb_d)]:
                    nc.scalar.dma_start(out=t[:], in_=d.ap())
        nc.scalar.dma_start(out=wk[:, 1], in_=wk_d.ap()[:, 1])
        nc.scalar.dma_start(out=wq[:, 1], in_=wq_d.ap()[:, 1])
        nc.scalar.dma_start(out=prt[:], in_=prt_d.ap())
        nc.scalar.dma_start(out=cos2[:], in_=cos_d.ap())
        nc.scalar.dma_start(out=sin2[:], in_=sin_d.ap())
        for mt in range(2, MT):
            nc.scalar.dma_start(out=wk[:, mt], in_=wk_d.ap()[:, mt])
            nc.scalar.dma_start(out=wq[:, mt], in_=wq_d.ap()[:, mt])
        for mt in range(MT):
            nc.gpsimd.dma_start(out=wo[:, mt], in_=wo_d.ap()[:, mt])

        ones_f = sb.tile([128, S], f32)
        nc.vector.memset(ones_f[:], 1.0)
        nc.vector.memset(
            vv[:, :, :, HEAD_DIM:HEAD_DIM + 1]
            .rearrange("p s h one -> p (s h one)"), 1.0)
        # only rows {0,32,64,96} of recq get real rowsums, but the bounce
        # DMA ships all 97 rows — initialize so the unused rows are defined
        nc.vector.memset(recq[:], 0.0)
        nc.vector.tensor_copy(ones64[:], ones_f[0:97, 0:HEAD_DIM])
        # tiny dummy exp: pulls the ~2.7us ACT table load into the DMA fill
        # and keeps the tile scheduler's model of the first real exps tight
        warm = sb.tile([1, 8], f32)
        with tc.high_priority():
            nc.scalar.activation(warm[:], ones_f[0:1, 0:8], AF.Exp)

        def rope_apply(dst, b, c2, ps, sbpool):
            # row-tile 0 only: RoPE on the first 64 flat channels (rows
            # 64-127 and the hg=1 core get identity via cos=1/sin=0).
            sl = slice(c2 * 512, (c2 + 1) * 512)
            sinp = sbpool.tile([128, 512], f16, tag="sinp", name="sinp")
            nc.vector.scalar_tensor_tensor(
                sinp[:], ps[:], b[:, 0:1], sin2[:, sl],
                op0=OP.add, op1=OP.mult)
            cosp = sbpool.tile([128, 512], f32, tag="cosp", name="cosp")
            nc.vector.scalar_tensor_tensor(
                cosp[:], ps[:], b[:, 0:1], cos2[:, sl],
                op0=OP.add, op1=OP.mult)
            pp = psp.tile([128, 512], f32, tag="ps", name="pp")
            nc.tensor.matmul(out=pp[:], lhsT=prt[:], rhs=sinp[:],
                             start=True, stop=True)
            nc.vector.tensor_tensor(dst[:, 0, sl], cosp[:], pp[:], op=OP.add)

        # ---- zipper chains (also inlined into the prologue tail) ------
        def chain_proj(dst, w, b, mt, half):
            sl = slice(half * 512, (half + 1) * 512)
            ps = psp.tile([128, 512], f32, tag="ps", name="ps")
            for kt in range(KT):
                nc.tensor.matmul(
                    out=ps[:], lhsT=w[:, mt, kt, :], rhs=xT[:, kt, sl],
                    start=(kt == 0), stop=(kt == KT - 1))
                if kt == KT - 1:
                    nc.vector.tensor_scalar(
                        dst[:, mt, sl], ps[:], b[:, mt:mt + 1],
                        None, op0=OP.add)
                yield

        # ---- prologue: mt0 K/Q chains + V, interleaved per kt ---------
        # so compute paces with the arriving per-kt DMA chunks.
        with tc.tile_pool(name="pkq", bufs=4, space="PSUM") as pkq, \
             tc.tile_pool(name="pvv", bufs=3, space="PSUM") as pvv, \
             tc.tile_pool(name="psb", bufs=2) as psb:

            def v_spill(st, ps):
                # bias add fused into the PSUM->SBUF spill (writes only the
                # 64 real V channels; col 64 stays the memset ones)
                nc.vector.tensor_tensor(
                    vv[:, st, :, 0:HEAD_DIM],
                    ps[:].rearrange("p (h d) -> p h d", h=HSH),
                    bvb[:], op=OP.add)

            kq = []
            for dst, w, b in ((kT, wk, bk), (qT, wq, bq)):
                for c2 in range(2):
                    kq.append((dst, w, b, c2,
                               pkq.tile([128, 512], f32, tag="kq",
                                        name="kq")))
            vps = [pvv.tile([128, DSH], f32, tag="vps", name="vps")
                   for _ in range(3)]
            for kt in range(KT):
                for dst, w, b, c2, ps in kq:
                    nc.tensor.matmul(
                        out=ps[:], lhsT=w[:, 0, kt, :],
                        rhs=xT[:, kt, c2 * 512:(c2 + 1) * 512],
                        start=(kt == 0), stop=(kt == KT - 1))
                for st in range(3):
                    nc.tensor.matmul(
                        out=vps[st][:],
                        lhsT=xT[:, kt, st * 128:(st + 1) * 128],
                        rhs=wv[:, kt, :],
                        start=(kt == 0), stop=(kt == KT - 1))
            for st in range(3):
                v_spill(st, vps[st])

            # interleave the rope chains (DVE-heavy) and the K-mt1 chains
            # with the remaining V chains so neither engine idles in the
            # DMA-paced prologue tail
            def v_chain(st):
                ps = pvv.tile([128, DSH], f32, tag="vps", name="vps")
                for kt in range(KT):
                    nc.tensor.matmul(
                        out=ps[:], lhsT=xT[:, kt, st * 128:(st + 1) * 128],
                        rhs=wv[:, kt, :],
                        start=(kt == 0), stop=(kt == KT - 1))
                v_spill(st, ps)

            def k1_chain(half):
                ps = pvv.tile([128, DSH], f32, tag="vps", name="kx")
                sl = slice(half * 512, (half + 1) * 512)
                for kt in range(KT):
                    nc.tensor.matmul(
                        out=ps[:], lhsT=wk[:, 1, kt, :], rhs=xT[:, kt, sl],
                        start=(kt == 0), stop=(kt == KT - 1))
                nc.vector.tensor_scalar(
                    kT[:, 1, sl], ps[:], bk[:, 1:2], None, op0=OP.add)

            for st in range(3, ST):
                if st - 3 < len(kq):
                    dst, w, b, c2, ps = kq[st - 3]
                    rope_apply(dst, b, c2, ps, psb)
                v_chain(st)
                if st == 3:
                    k1_chain(0)
                if st == 4:
                    k1_chain(1)
                if st == 5:  # qT mt1 chains fill the DMA-paced slack here
                    for _ in chain_proj(qT, wq, bq, 1, 0):
                        pass
                if st == 6:
                    for _ in chain_proj(qT, wq, bq, 1, 1):
                        pass

        # (chain_proj is defined before the prologue so the prologue can
        # inline two qT-mt1 chains into its DMA-paced tail)

        def chain_out(qt, n2, pool, tag, obpool, copy_eng, dma_eng=None):
            nsl = slice(n2 * 512, (n2 + 1) * 512)
            ps = pool.tile([128, 512], f32, tag=tag, name="ps3")
            for mt in range(MT):
                nc.tensor.matmul(
                    out=ps[:], lhsT=attU[:, mt, qt * 128:(qt + 1) * 128],
                    rhs=wo[:, mt, nsl],
                    start=(mt == 0), stop=(mt == MT - 1))
                if mt == MT - 1:
                    ob = obpool.tile([128, 512], f16, tag="ob", name="ob")
                    if copy_eng == "act":
                        nc.scalar.activation(ob[:], ps[:], AF.Copy)
                    else:
                        nc.vector.tensor_copy(ob[:], ps[:])
                    eng = dma_eng
                    if eng is None:
                        eng = nc.sync if (qt + n2) % 2 == 0 else nc.scalar
                    eng.dma_start(
                        out=out_d.ap()[qt * 128:(qt + 1) * 128, nsl],
                        in_=ob[:])
                yield

        # ---- phase 2+3: attention with zippered projections/output ----
        with tc.tile_pool(name="psc", bufs=2, space="PSUM") as psc, \
             tc.tile_pool(name="pat0", bufs=2, space="PSUM") as pat0, \
             tc.tile_pool(name="pat1", bufs=1, space="PSUM") as pat1, \
             tc.tile_pool(name="p2sb", bufs=3) as p2sb, \
             tc.tile_pool(name="p2r", bufs=2) as p2r:

            def emit_scores(mt, kt, c2):
                sch = psc.tile([128, S], f32, tag="sch", name="sch")
                qsl = slice(c2 * 512, (c2 + 1) * 512)
                for hh in range(2):
                    ph = hh * 64
                    nc.tensor.matmul(
                        out=sch[:, hh * 512:hh * 512 + 512],
                        lhsT=kT[ph:ph + 64, mt, kt * 128:(kt + 1) * 128],
                        rhs=qT[ph:ph + 64, mt, qsl],
                        start=True, stop=True, tile_position=(ph, 0))
                return sch

            def normalize_bounce(mt, c2):
                # rows 32*(2hh+c2) of recq hold 1/rowsum for head (2mt+hh),
                # q-half c2.  DRAM bounce partition-broadcasts each row; the
                # multiply runs on the otherwise-idle GpSimd engine.
                nc.gpsimd.dma_start(out=recd.ap()[:, mt, :],
                                    in_=recq[:, mt, :])
                qsl = slice(c2 * 512, (c2 + 1) * 512)
                for hh in range(2):
                    ph, r = hh * 64, 32 * (hh * 2 + c2)
                    rb = p2r.tile([128, 512], f32, tag="rb", name="rb")
                    nc.gpsimd.dma_start(
                        out=rb[ph:ph + 64],
                        in_=recd.ap()[r:r + 1, mt, :].partition_broadcast(64))
                    nc.gpsimd.tensor_tensor(
                        attU[ph:ph + 64, mt, qsl], attU[ph:ph + 64, mt, qsl],
                        rb[ph:ph + 64], op=OP.mult)

            def normalize_pe(mt, c2):
                # PE ones-matmul broadcast (for the final tile where the
                # DMA bounce latency would sit exposed)
                qsl = slice(c2 * 512, (c2 + 1) * 512)
                rbps = psp.tile([128, 512], f32, tag="ps", name="rbps")
                for hh in range(2):
                    ph, r = hh * 64, 32 * (hh * 2 + c2)
                    nc.tensor.matmul(
                        out=rbps[ph:ph + 64, :], lhsT=ones64[r:r + 1, :],
                        rhs=recq[r:r + 1, mt, :],
                        start=True, stop=True, tile_position=(r, ph))
                for hh in range(2):
                    ph = hh * 64
                    nc.vector.tensor_tensor(
                        attU[ph:ph + 64, mt, qsl], attU[ph:ph + 64, mt, qsl],
                        rbps[ph:ph + 64, :], op=OP.mult)

            def gen_c2_0():
                # order is deadline-driven: each chain must be fully
                # EMITTED before the lookahead scores emission that reads
                # it (program order defines Tile's deps); K-mt1 and Q-mt1
                # chains ran in the prologue
                yield from chain_proj(qT, wq, bq, 2, 0)
                yield from chain_proj(kT, wk, bk, 2, 0)
                yield from chain_proj(kT, wk, bk, 2, 1)
                yield from chain_proj(qT, wq, bq, 3, 0)
                yield from chain_proj(kT, wk, bk, 3, 0)
                yield from chain_proj(kT, wk, bk, 3, 1)
                while True:
                    yield

            def gen_c2_1():
                yield from chain_proj(qT, wq, bq, 2, 1)
                yield from chain_proj(qT, wq, bq, 3, 1)
                for qt in range(4):
                    for n2 in range(2):
                        yield from chain_out(qt, n2, psp, "ps", p2sb, "dve")
                while True:
                    yield

            # ---- flat 64-group software pipeline --------------------------
            # group g = (c2, mt, kt); exp/scores run one group AHEAD of the
            # PV/rowsum consumers, crossing mt and c2 boundaries, so neither
            # the PE nor ACT ever drains at a boundary.
            ZIPN = {0: (3, 2, 1, 1), 1: (1, 2, 2, 2)}
            groups = [(c2, mt, kt) for c2 in range(2) for mt in range(MT)
                      for kt in range(ST)]
            gens = {0: gen_c2_0(), 1: gen_c2_1()}
            NG = len(groups)
            pts, ats = {}, {}
            sch = {0: emit_scores(0, 0, 0)}
            for g in range(NG + 1):
                if g < NG:
                    c2, mt, kt = groups[g]
                    pts[g] = p2sb.tile([128, S], f16, tag="pt", name="pt")
                    nc.scalar.activation(
                        pts[g][:], sch.pop(g)[:], AF.Exp,
                        bias=maskb[:, kt:kt + 1], scale=0.125)
                    if g + 1 < NG:
                        nc2, nmt, nkt = groups[g + 1]
                        sch[g + 1] = emit_scores(nmt, nkt, nc2)
                    for _ in range(ZIPN[c2][mt]):
                        next(gens[c2])
                if g >= 1:
                    c2, mt, kt = groups[g - 1]
                    first, last = (kt == 0), (kt == ST - 1)
                    if first:
                        ats[mt, c2] = (
                            pat0.tile([128, 512], f32, tag="at0", name="at0"),
                            pat1.tile([128, 512], f32, tag="at1", name="at1"))
                    atx = ats[mt, c2]
                    pt = pts.pop(g - 1)
                    for hh in range(2):  # PV pair, rowsum row merged (M=65)
                        nc.tensor.matmul(
                            out=atx[hh][0:65, :],
                            lhsT=vv[:, kt, mt * 2 + hh, :],
                            rhs=pt[:, hh * 512:hh * 512 + 512],
                            start=first, stop=last)
                    if last:
                        # epilogue.  recip runs via partition 0 (the
                        # custom-DVE recip is base-0-only).  Mid-loop the
                        # at1 readers go first (its ring is single-
                        # buffered; the next group's PV waits on them);
                        # in the final epilogue the recip chain goes
                        # first instead -- it gates the tail's normalize.
                        qsl = slice(c2 * 512, (c2 + 1) * 512)
                        final = (c2 == 1 and mt == MT - 1)
                        nc.vector.tensor_copy(
                            rss2[0:1, 1], atx[1][64:65, :])
                        if not final:
                            nc.vector.tensor_copy(
                                attU[64:128, mt, qsl], atx[1][0:64, :])
                        nc.vector.tensor_copy(
                            rss2[0:1, 0], atx[0][64:65, :])
                        nc.vector.reciprocal_approx_fast(
                            recf2[0:1, :, :], rss2[0:1, :, :])
                        for hh in range(2):
                            r = 32 * (hh * 2 + c2)
                            nc.vector.tensor_copy(
                                recq[r:r + 1, mt, :], recf2[0:1, hh])
                        if final:
                            nc.vector.tensor_copy(
                                attU[64:128, mt, qsl], atx[1][0:64, :])
                        nc.vector.tensor_copy(
                            attU[0:64, mt, qsl], atx[0][0:64, :])
                        if (c2 == 0 and mt < MT - 1) or \
                           (c2 == 1 and mt < MT - 2):
                            normalize_bounce(mt, c2)
                        else:
                            normalize_pe(mt, c2)

            # ---- tail: output projection for q-half 1 -----------------
            pools = [(psp, "ps"), (pat0, "at0"), (pat1, "at1")]
            dma_engs = [nc.sync, nc.scalar, nc.gpsimd]
            for i, (qt, n2) in enumerate(
                    (q, n) for q in range(4, 8) for n in range(2)):
                pool, tag = pools[i % 3]
                eng = "act" if i % 2 == 0 else "dve"
                for _ in chain_out(qt, n2, pool, tag, p2sb, eng,
                                   dma_eng=dma_engs[i % 3]):
                    pass

    nc.compile()
    return nc


def _get_nc():
    if "nc" not in _CACHE:
        _CACHE["nc"] = _build()
    return _CACHE["nc"]


def _prep_inputs(x, mask, freqs, Wq, bq, Wk, bk, Wv, bv, Wo, bo):
    f = np.asarray(freqs, np.float32)[0]              # [S, HEAD_DIM]
    # reference rotates only the first rot_dim=64 channels of the FLAT
    # inner dim -> rows 0-63 of row-tile 0 on the hg=0 core; everything
    # else is identity (cos=1, sin=0).
    cos2 = np.ones((128, S), np.float32)
    sin2 = np.zeros((128, S), np.float32)
    cos2[0:HEAD_DIM] = np.cos(f.T)
    sin2[0:HEAD_DIM] = np.sin(f.T)
    ident = np.ones((128, S), np.float32)
    identz = np.zeros((128, S), np.float32)

    prt = np.zeros((128, 128), np.float16)            # P_rot^T
    i = np.arange(0, 128, 2)
    prt[i + 1, i] = -1.0                              # P_rot[2i, 2i+1] = -1
    prt[i, i + 1] = 1.0                               # P_rot[2i+1, 2i] = +1

    def lhsT_w(w):                       # [DIM, DSH] -> [128, MT, KT, 128]
        return np.ascontiguousarray(
            w.reshape(KT, 128, MT, 128).transpose(1, 2, 0, 3)
        ).astype(np.float16)

    def col(b):                                       # [DSH] -> [128, MT]
        return np.ascontiguousarray(b.reshape(MT, 128).T.astype(np.float32))

    in_maps = []
    for b in range(B):
        xT = np.ascontiguousarray(
            np.asarray(x[b], np.float32).T.reshape(KT, 128, S)
            .transpose(1, 0, 2)).astype(np.float16)
        m = np.asarray(mask[b])
        maskb = np.ascontiguousarray(
            np.where(m, 0.0, MASK_NEG).astype(np.float32).reshape(ST, 128).T)
        for hg in range(HG):
            dsl = slice(hg * DSH, (hg + 1) * DSH)
            in_maps.append({
                "xT": xT,
                "wq": lhsT_w(np.asarray(Wq, np.float32)[:, dsl]),
                "wk": lhsT_w(np.asarray(Wk, np.float32)[:, dsl]),
                "wv": np.ascontiguousarray(
                    np.asarray(Wv, np.float32)[:, dsl]
                    .reshape(KT, 128, DSH).transpose(1, 0, 2)).astype(np.float16),
                "wo": np.ascontiguousarray(
                    np.asarray(Wo, np.float32)[dsl, :]
                    .reshape(MT, 128, DIM).transpose(1, 0, 2)).astype(np.float16),
                "bq": col(np.asarray(bq, np.float32)[dsl]),
                "bk": col(np.asarray(bk, np.float32)[dsl]),
                "bv": np.asarray(bv, np.float32)[None, dsl]
                    .astype(np.float16).copy(),
                "cos2": cos2 if hg == 0 else ident,
                "sin2": sin2 if hg == 0 else identz,
                "prt": prt,
                "maskb": maskb,
            })
    return in_maps


def run(trace=False, **inputs):
    from concourse import bass_utils
    if trace:
        _install_ntff_hook()
    nc = _get_nc()
    in_maps = _prep_inputs(**inputs)
    res = bass_utils.run_bass_kernel_spmd(
        nc, in_maps, core_ids=list(range(NCORES)), trace=trace)
    mask = np.asarray(inputs["mask"])
    bo = np.asarray(inputs["bo"], np.float32)
    out = np.empty((B, S, DIM), np.float32)
    for b in range(B):
        p = (res.results[2 * b]["out"].astype(np.float32)
             + res.results[2 * b + 1]["out"].astype(np.float32) + bo)
        out[b] = np.where(mask[b][:, None], p, 0.0)
    return out, res


def kernel(**inputs):
    out, _ = run(trace=False, **inputs)
    return out


def _install_ntff_hook():
    """Register the axon NTFF profiling hook missing from the antenv stub."""
    import sys, types
    try:
        import antenv.axon_hooks  # noqa: F401
        return
    except ImportError:
        pass
    from trn_agent_boot.trn_boot import _ntff_profile_via_ctypes
    hook = _ntff_profile_via_ctypes('/opt/axon/libaxon_pjrt.so')
    mod = types.ModuleType('antenv.axon_hooks')
    mod.get_axon_ntff_profile_hook = lambda: hook
    mod.set_axon_ntff_profile_hook = lambda h: None
    sys.modules['antenv.axon_hooks'] = mod



# revision 56
# speedup vs baseline: 1.0560x; 1.0240x over previous
"""Trainium2 Bass kernel for nn_Attention (B=4, S=1024, DIM=1024, H=16, Dh=64).

Sharding: 8 cores = 4 batches x 2 head-groups (8 heads / 512 inner channels
each).  Each core computes q/k/v projections for its head shard, RoPE,
attention, and a partial output projection (its rows of Wo); the host sums
the two head-group partials per batch, adds the output bias and applies the
query mask (the cheap elementwise epilogue of the unshard step).

Device dataflow (per core), matmul operands in fp16 (fp32 PSUM accumulate):
  inputs stream in per-kt chunks on three HW DMA queues; the first 7
  projection chains (K/Q row-tile 0 + V st 0-2) interleave per-kt so the PE
  starts ~1us in and never waits on HBM.
  Q^T,K^T = W^T @ x^T  (bias added on the PSUM->SBUF pass); RoPE on the
  first 64 flat channels (host sends cos=1/sin=0 elsewhere).
  Phase 2 runs q-half (c2) OUTER, head-pair row-tile (mt) inner:
    scores^T[k,q] = K_h @ Q_h^T      (row-tiled pair)
    P^T = exp(scores^T/8 + maskb[k]) (one ACT op per (mt,kt) over 2h x 512q)
    attn^T += [V^T | 1] @ P^T        (M=65: softmax rowsum lands at output
                                      row 64, replacing separate M=1
                                      rowsum matmuls)
  as one flat 64-group software pipeline (PV lags the exp by one group,
  crossing mt/c2 boundaries) with the mt1-3 K/Q projection matmuls
  zippered into the q-half-0 groups and the q-half-0 output-projection
  chains zippered into the q-half-1 groups, so the PE stays dense while
  ACT streams exps.  NOTE: zipper chains must be fully EMITTED before the
  lookahead scores emission that reads their output (emission order
  defines Tile's dependencies).
  Normalization: approx-reciprocal of rowsums staged through partition 0
  (the custom-DVE recip only works on base-partition-0 SBUF operands --
  HW-probed); DRAM-bounce partition broadcast + gpsimd multiply wherever
  slack allows, PE ones-matmul broadcast + DVE multiply for the last
  tiles (SWDGE bounce latency would sit on the tail).
  Tail: the 8 q-half-1 output chains are software-pipelined over 3 PSUM
  rings -- all mt0-2 matmuls issue immediately; only each chain's mt3
  matmul gates on the final normalize.
  out[q,:] = attn_norm^T.T @ Wo_shard, written as fp16; host adds bo and
  masks.
"""

import numpy as np

B, S, DIM, HEADS, HEAD_DIM = 4, 1024, 1024, 16, 64
INNER = HEADS * HEAD_DIM
HG = 2                      # head groups (tensor-parallel shards)
DSH = INNER // HG           # 512 inner channels per core
HSH = HEADS // HG           # 8 heads per core
NCORES = B * HG
KT = DIM // 128             # 8 contraction tiles
MT = DSH // 128             # 4 row tiles (head pairs)
ST = S // 128               # 8 seq tiles
MASK_NEG = -80.0

_CACHE = {}


def _build():
    import concourse.tile as tile
    from concourse import bacc, mybir

    f32 = mybir.dt.float32
    f16 = mybir.dt.float16
    AF = mybir.ActivationFunctionType
    OP = mybir.AluOpType

    nc = bacc.Bacc("TRN2", target_bir_lowering=False, debug=False)

    xT_d = nc.dram_tensor("xT", [128, KT, S], f16, kind="ExternalInput")
    wq_d = nc.dram_tensor("wq", [128, MT, KT, 128], f16, kind="ExternalInput")
    wk_d = nc.dram_tensor("wk", [128, MT, KT, 128], f16, kind="ExternalInput")
    wv_d = nc.dram_tensor("wv", [128, KT, DSH], f16, kind="ExternalInput")
    wo_d = nc.dram_tensor("wo", [128, MT, DIM], f16, kind="ExternalInput")
    bq_d = nc.dram_tensor("bq", [128, MT], f32, kind="ExternalInput")
    bk_d = nc.dram_tensor("bk", [128, MT], f32, kind="ExternalInput")
    bv_d = nc.dram_tensor("bv", [1, DSH], f16, kind="ExternalInput")
    cos_d = nc.dram_tensor("cos2", [128, S], f32, kind="ExternalInput")
    sin_d = nc.dram_tensor("sin2", [128, S], f32, kind="ExternalInput")
    prt_d = nc.dram_tensor("prt", [128, 128], f16, kind="ExternalInput")
    maskb_d = nc.dram_tensor("maskb", [128, ST], f32, kind="ExternalInput")
    out_d = nc.dram_tensor("out", [S, DIM], f16, kind="ExternalOutput")
    recd = nc.dram_tensor("recd", [97, MT, 512], f16)

    with tile.TileContext(nc) as tc, \
         tc.tile_pool(name="sb", bufs=1) as sb, \
         tc.tile_pool(name="psp", bufs=1, space="PSUM") as psp:

        # ---- persistent SBUF ------------------------------------------
        xT = sb.tile([128, KT, S], f16)
        wq = sb.tile([128, MT, KT, 128], f16)
        wk = sb.tile([128, MT, KT, 128], f16)
        wv = sb.tile([128, KT, DSH], f16)
        wo = sb.tile([128, MT, DIM], f16)
        bq = sb.tile([128, MT], f32)
        bk = sb.tile([128, MT], f32)
        bvb = sb.tile([128, HSH, HEAD_DIM], f16)
        cos2 = sb.tile([128, S], f32)
        sin2 = sb.tile([128, S], f32)
        prt = sb.tile([128, 128], f16)
        maskb = sb.tile([128, ST], f32)
        qT = sb.tile([128, MT, S], f16)
        kT = sb.tile([128, MT, S], f16)
        # V with a ones column appended per head: the PV matmul (M=65)
        # then produces the softmax rowsum at output row 64 for free,
        # replacing the 128 M=1 rowsum matmuls (~20us of PE streaming).
        vv = sb.tile([128, ST, HSH, HEAD_DIM + 1], f16)
        attU = sb.tile([128, MT, S], f16)
        # rowsum staging at partition 0: reciprocal_approx_fast only works
        # with base-partition-0 SBUF operands (HW-probed; base 64 or PSUM
        # input reads garbage)
        rss2 = sb.tile([1, 2, 512], f32)
        recf2 = sb.tile([1, 2, 512], f32)
        recq = sb.tile([97, MT, 512], f16)
        ones64 = sb.tile([97, HEAD_DIM], f16)

        # ---- input DMAs: per-kt chunks on three HW DMA queues ---------
        # sync+vector: xT halves; gpsimd: wv; scalar: wk/wq (mt0 first),
        # smalls, rest, wo.
        nc.gpsimd.dma_start(
            out=bvb[:].rearrange("p h d -> p (h d)"),
            in_=bv_d.ap()[0:1, :].partition_broadcast(128))
        for kt in range(KT):
            nc.sync.dma_start(out=xT[:, kt, 0:512],
                              in_=xT_d.ap()[:, kt, 0:512])
            nc.sync.dma_start(out=xT[:, kt, 512:1024],
                              in_=xT_d.ap()[:, kt, 512:1024])
            nc.gpsimd.dma_start(out=wv[:, kt], in_=wv_d.ap()[:, kt])
            nc.scalar.dma_start(out=wk[:, 0, kt], in_=wk_d.ap()[:, 0, kt])
            nc.scalar.dma_start(out=wq[:, 0, kt], in_=wq_d.ap()[:, 0, kt])
            if kt == 1:
                for t, d in [(bq, bq_d), (bk, bk_d), (maskb, maskb_d)]:
                    nc.scalar.dma_start(out=t[:], in_=d.ap())
        nc.scalar.dma_start(out=prt[:], in_=prt_d.ap())
        nc.scalar.dma_start(out=cos2[:], in_=cos_d.ap())
        nc.scalar.dma_start(out=sin2[:], in_=sin_d.ap())
        for mt in range(1, MT):
            nc.scalar.dma_start(out=wk[:, mt], in_=wk_d.ap()[:, mt])
            nc.scalar.dma_start(out=wq[:, mt], in_=wq_d.ap()[:, mt])
        for mt in range(MT):
            nc.scalar.dma_start(out=wo[:, mt], in_=wo_d.ap()[:, mt])

        ones_f = sb.tile([128, S], f32)
        nc.vector.memset(ones_f[:], 1.0)
        nc.vector.memset(
            vv[:, :, :, HEAD_DIM:HEAD_DIM + 1]
            .rearrange("p s h one -> p (s h one)"), 1.0)
        # only rows {0,32,64,96} of recq get real rowsums, but the bounce
        # DMA ships all 97 rows — initialize so the unused rows are defined
        nc.vector.memset(recq[:], 0.0)
        nc.vector.tensor_copy(ones64[:], ones_f[0:97, 0:HEAD_DIM])
        # tiny dummy exp: pulls the ~2.7us ACT table load into the DMA fill
        # and keeps the tile scheduler's model of the first real exps tight
        warm = sb.tile([1, 8], f32)
        with tc.high_priority():
            nc.scalar.activation(warm[:], ones_f[0:1, 0:8], AF.Exp)

        def rope_apply(dst, b, c2, ps, sbpool):
            # row-tile 0 only: RoPE on the first 64 flat channels (rows
            # 64-127 and the hg=1 core get identity via cos=1/sin=0).
            sl = slice(c2 * 512, (c2 + 1) * 512)
            sinp = sbpool.tile([128, 512], f16, tag="sinp", name="sinp")
            nc.vector.scalar_tensor_tensor(
                sinp[:], ps[:], b[:, 0:1], sin2[:, sl],
                op0=OP.add, op1=OP.mult)
            cosp = sbpool.tile([128, 512], f32, tag="cosp", name="cosp")
            nc.vector.scalar_tensor_tensor(
                cosp[:], ps[:], b[:, 0:1], cos2[:, sl],
                op0=OP.add, op1=OP.mult)
            pp = psp.tile([128, 512], f32, tag="ps", name="pp")
            nc.tensor.matmul(out=pp[:], lhsT=prt[:], rhs=sinp[:],
                             start=True, stop=True)
            nc.vector.tensor_tensor(dst[:, 0, sl], cosp[:], pp[:], op=OP.add)

        # ---- prologue: mt0 K/Q chains + V, interleaved per kt ---------
        # so compute paces with the arriving per-kt DMA chunks.
        with tc.tile_pool(name="pkq", bufs=4, space="PSUM") as pkq, \
             tc.tile_pool(name="pvv", bufs=3, space="PSUM") as pvv, \
             tc.tile_pool(name="psb", bufs=2) as psb:

            def v_spill(st, ps):
                # bias add fused into the PSUM->SBUF spill (writes only the
                # 64 real V channels; col 64 stays the memset ones)
                nc.vector.tensor_tensor(
                    vv[:, st, :, 0:HEAD_DIM],
                    ps[:].rearrange("p (h d) -> p h d", h=HSH),
                    bvb[:], op=OP.add)

            kq = []
            for dst, w, b in ((kT, wk, bk), (qT, wq, bq)):
                for c2 in range(2):
                    kq.append((dst, w, b, c2,
                               pkq.tile([128, 512], f32, tag="kq",
                                        name="kq")))
            vps = [pvv.tile([128, DSH], f32, tag="vps", name="vps")
                   for _ in range(3)]
            for kt in range(KT):
                for dst, w, b, c2, ps in kq:
                    nc.tensor.matmul(
                        out=ps[:], lhsT=w[:, 0, kt, :],
                        rhs=xT[:, kt, c2 * 512:(c2 + 1) * 512],
                        start=(kt == 0), stop=(kt == KT - 1))
                for st in range(3):
                    nc.tensor.matmul(
                        out=vps[st][:],
                        lhsT=xT[:, kt, st * 128:(st + 1) * 128],
                        rhs=wv[:, kt, :],
                        start=(kt == 0), stop=(kt == KT - 1))
            for st in range(3):
                v_spill(st, vps[st])

            # interleave the rope chains (DVE-heavy) and the K-mt1 chains
            # with the remaining V chains so neither engine idles in the
            # DMA-paced prologue tail
            def v_chain(st):
                ps = pvv.tile([128, DSH], f32, tag="vps", name="vps")
                for kt in range(KT):
                    nc.tensor.matmul(
                        out=ps[:], lhsT=xT[:, kt, st * 128:(st + 1) * 128],
                        rhs=wv[:, kt, :],
                        start=(kt == 0), stop=(kt == KT - 1))
                v_spill(st, ps)

            def k1_chain(half):
                ps = pvv.tile([128, DSH], f32, tag="vps", name="kx")
                sl = slice(half * 512, (half + 1) * 512)
                for kt in range(KT):
                    nc.tensor.matmul(
                        out=ps[:], lhsT=wk[:, 1, kt, :], rhs=xT[:, kt, sl],
                        start=(kt == 0), stop=(kt == KT - 1))
                nc.vector.tensor_scalar(
                    kT[:, 1, sl], ps[:], bk[:, 1:2], None, op0=OP.add)

            for st in range(3, ST):
                if st - 3 < len(kq):
                    dst, w, b, c2, ps = kq[st - 3]
                    rope_apply(dst, b, c2, ps, psb)
                v_chain(st)
                if st == 3:
                    k1_chain(0)
                if st == 4:
                    k1_chain(1)

        # ---- zipper chains (run inside phase-2 groups) ----------------
        def chain_proj(dst, w, b, mt, half):
            sl = slice(half * 512, (half + 1) * 512)
            ps = psp.tile([128, 512], f32, tag="ps", name="ps")
            for kt in range(KT):
                nc.tensor.matmul(
                    out=ps[:], lhsT=w[:, mt, kt, :], rhs=xT[:, kt, sl],
                    start=(kt == 0), stop=(kt == KT - 1))
                if kt == KT - 1:
                    nc.vector.tensor_scalar(
                        dst[:, mt, sl], ps[:], b[:, mt:mt + 1],
                        None, op0=OP.add)
                yield

        def chain_out(qt, n2, pool, tag, obpool, copy_eng, dma_eng=None):
            nsl = slice(n2 * 512, (n2 + 1) * 512)
            ps = pool.tile([128, 512], f32, tag=tag, name="ps3")
            for mt in range(MT):
                nc.tensor.matmul(
                    out=ps[:], lhsT=attU[:, mt, qt * 128:(qt + 1) * 128],
                    rhs=wo[:, mt, nsl],
                    start=(mt == 0), stop=(mt == MT - 1))
                if mt == MT - 1:
                    ob = obpool.tile([128, 512], f16, tag="ob", name="ob")
                    if copy_eng == "act":
                        nc.scalar.activation(ob[:], ps[:], AF.Copy)
                    else:
                        nc.vector.tensor_copy(ob[:], ps[:])
                    eng = dma_eng
                    if eng is None:
                        eng = nc.sync if (qt + n2) % 2 == 0 else nc.scalar
                    eng.dma_start(
                        out=out_d.ap()[qt * 128:(qt + 1) * 128, nsl],
                        in_=ob[:])
                yield

        # ---- phase 2+3: attention with zippered projections/output ----
        with tc.tile_pool(name="psc", bufs=2, space="PSUM") as psc, \
             tc.tile_pool(name="pat0", bufs=2, space="PSUM") as pat0, \
             tc.tile_pool(name="pat1", bufs=1, space="PSUM") as pat1, \
             tc.tile_pool(name="p2sb", bufs=3) as p2sb, \
             tc.tile_pool(name="p2r", bufs=2) as p2r:

            def emit_scores(mt, kt, c2):
                sch = psc.tile([128, S], f32, tag="sch", name="sch")
                qsl = slice(c2 * 512, (c2 + 1) * 512)
                for hh in range(2):
                    ph = hh * 64
                    nc.tensor.matmul(
                        out=sch[:, hh * 512:hh * 512 + 512],
                        lhsT=kT[ph:ph + 64, mt, kt * 128:(kt + 1) * 128],
                        rhs=qT[ph:ph + 64, mt, qsl],
                        start=True, stop=True, tile_position=(ph, 0))
                return sch

            def normalize_bounce(mt, c2):
                # rows 32*(2hh+c2) of recq hold 1/rowsum for head (2mt+hh),
                # q-half c2.  DRAM bounce partition-broadcasts each row; the
                # multiply runs on the otherwise-idle GpSimd engine.
                nc.gpsimd.dma_start(out=recd.ap()[:, mt, :],
                                    in_=recq[:, mt, :])
                qsl = slice(c2 * 512, (c2 + 1) * 512)
                for hh in range(2):
                    ph, r = hh * 64, 32 * (hh * 2 + c2)
                    rb = p2r.tile([128, 512], f32, tag="rb", name="rb")
                    nc.gpsimd.dma_start(
                        out=rb[ph:ph + 64],
                        in_=recd.ap()[r:r + 1, mt, :].partition_broadcast(64))
                    nc.gpsimd.tensor_tensor(
                        attU[ph:ph + 64, mt, qsl], attU[ph:ph + 64, mt, qsl],
                        rb[ph:ph + 64], op=OP.mult)

            def normalize_pe(mt, c2):
                # PE ones-matmul broadcast (for the final tile where the
                # DMA bounce latency would sit exposed)
                qsl = slice(c2 * 512, (c2 + 1) * 512)
                rbps = psp.tile([128, 512], f32, tag="ps", name="rbps")
                for hh in range(2):
                    ph, r = hh * 64, 32 * (hh * 2 + c2)
                    nc.tensor.matmul(
                        out=rbps[ph:ph + 64, :], lhsT=ones64[r:r + 1, :],
                        rhs=recq[r:r + 1, mt, :],
                        start=True, stop=True, tile_position=(r, ph))
                for hh in range(2):
                    ph = hh * 64
                    nc.vector.tensor_tensor(
                        attU[ph:ph + 64, mt, qsl], attU[ph:ph + 64, mt, qsl],
                        rbps[ph:ph + 64, :], op=OP.mult)

            def gen_c2_0():
                # order is deadline-driven: each chain must be fully
                # EMITTED before the lookahead scores emission that reads
                # it (program order defines Tile's deps); K-mt1 chains ran
                # in the prologue
                yield from chain_proj(qT, wq, bq, 1, 0)
                yield from chain_proj(qT, wq, bq, 2, 0)
                yield from chain_proj(kT, wk, bk, 2, 0)
                yield from chain_proj(kT, wk, bk, 2, 1)
                yield from chain_proj(qT, wq, bq, 3, 0)
                yield from chain_proj(kT, wk, bk, 3, 0)
                yield from chain_proj(kT, wk, bk, 3, 1)
                yield from chain_proj(qT, wq, bq, 1, 1)
                while True:
                    yield

            def gen_c2_1():
                yield from chain_proj(qT, wq, bq, 2, 1)
                yield from chain_proj(qT, wq, bq, 3, 1)
                for qt in range(4):
                    for n2 in range(2):
                        yield from chain_out(qt, n2, psp, "ps", p2sb, "dve")
                while True:
                    yield

            # ---- flat 64-group software pipeline --------------------------
            # group g = (c2, mt, kt); exp/scores run one group AHEAD of the
            # PV/rowsum consumers, crossing mt and c2 boundaries, so neither
            # the PE nor ACT ever drains at a boundary.
            ZIPN = {0: (3, 2, 2, 1), 1: (1, 2, 2, 2)}
            groups = [(c2, mt, kt) for c2 in range(2) for mt in range(MT)
                      for kt in range(ST)]
            gens = {0: gen_c2_0(), 1: gen_c2_1()}
            NG = len(groups)
            pts, ats = {}, {}
            sch = {0: emit_scores(0, 0, 0)}
            for g in range(NG + 1):
                if g < NG:
                    c2, mt, kt = groups[g]
                    pts[g] = p2sb.tile([128, S], f16, tag="pt", name="pt")
                    nc.scalar.activation(
                        pts[g][:], sch.pop(g)[:], AF.Exp,
                        bias=maskb[:, kt:kt + 1], scale=0.125)
                    if g + 1 < NG:
                        nc2, nmt, nkt = groups[g + 1]
                        sch[g + 1] = emit_scores(nmt, nkt, nc2)
                    for _ in range(ZIPN[c2][mt]):
                        next(gens[c2])
                if g >= 1:
                    c2, mt, kt = groups[g - 1]
                    first, last = (kt == 0), (kt == ST - 1)
                    if first:
                        ats[mt, c2] = (
                            pat0.tile([128, 512], f32, tag="at0", name="at0"),
                            pat1.tile([128, 512], f32, tag="at1", name="at1"))
                    atx = ats[mt, c2]
                    pt = pts.pop(g - 1)
                    for hh in range(2):  # PV pair, rowsum row merged (M=65)
                        nc.tensor.matmul(
                            out=atx[hh][0:65, :],
                            lhsT=vv[:, kt, mt * 2 + hh, :],
                            rhs=pt[:, hh * 512:hh * 512 + 512],
                            start=first, stop=last)
                    if last:
                        # epilogue.  recip runs via partition 0 (the
                        # custom-DVE recip is base-0-only, SBUF-only).
                        # Mid-loop the at1 readers go first (its ring is
                        # single-buffered; the next group's PV waits on
                        # them); in the final epilogue the recip chain
                        # goes first -- it gates the tail's normalize.
                        qsl = slice(c2 * 512, (c2 + 1) * 512)
                        final = (c2 == 1 and mt == MT - 1)
                        nc.vector.tensor_copy(
                            rss2[0:1, 1], atx[1][64:65, :])
                        if not final:
                            nc.vector.tensor_copy(
                                attU[64:128, mt, qsl], atx[1][0:64, :])
                        nc.vector.tensor_copy(
                            rss2[0:1, 0], atx[0][64:65, :])
                        nc.vector.reciprocal_approx_fast(
                            recf2[0:1, :, :], rss2[0:1, :, :])
                        for hh in range(2):
                            r = 32 * (hh * 2 + c2)
                            nc.vector.tensor_copy(
                                recq[r:r + 1, mt, :], recf2[0:1, hh])
                        if final:
                            nc.vector.tensor_copy(
                                attU[64:128, mt, qsl], atx[1][0:64, :])
                        nc.vector.tensor_copy(
                            attU[0:64, mt, qsl], atx[0][0:64, :])
                        if (c2 == 0 and mt < MT - 1) or \
                           (c2 == 1 and mt < MT - 2):
                            normalize_bounce(mt, c2)
                        else:
                            normalize_pe(mt, c2)

            # ---- tail: output projection for q-half 1 -----------------
            # software-pipelined over 3 PSUM rings: every chain's mt0-2
            # MMs are ready the moment the loop ends (attU c2=1 mt0-2
            # normalized much earlier); only the mt3 MM gates on the
            # final normalize.  Emitting all ready MMs first keeps the
            # in-order PE queue busy while the last epilogue completes.
            pools = [(psp, "ps"), (pat0, "at0"), (pat1, "at1")]
            dma_engs = [nc.sync, nc.scalar, nc.gpsimd]
            chains = [(q, n) for q in range(4, 8) for n in range(2)]
            pend = {}

            def tail_finish(i):
                qt, n2, ps = pend.pop(i)
                nsl = slice(n2 * 512, (n2 + 1) * 512)
                nc.tensor.matmul(
                    out=ps[:], lhsT=attU[:, 3, qt * 128:(qt + 1) * 128],
                    rhs=wo[:, 3, nsl], start=False, stop=True)
                ob = p2sb.tile([128, 512], f16, tag="ob", name="ob")
                if i % 2 == 0:
                    nc.scalar.activation(ob[:], ps[:], AF.Copy)
                else:
                    nc.vector.tensor_copy(ob[:], ps[:])
                dma_engs[i % 3].dma_start(
                    out=out_d.ap()[qt * 128:(qt + 1) * 128, nsl], in_=ob[:])

            for i, (qt, n2) in enumerate(chains):
                if i >= 3:
                    tail_finish(i - 3)
                pool, tag = pools[i % 3]
                ps = pool.tile([128, 512], f32, tag=tag, name="ps3")
                nsl = slice(n2 * 512, (n2 + 1) * 512)
                for mt in range(3):
                    nc.tensor.matmul(
                        out=ps[:],
                        lhsT=attU[:, mt, qt * 128:(qt + 1) * 128],
                        rhs=wo[:, mt, nsl], start=(mt == 0), stop=False)
                pend[i] = (qt, n2, ps)
            for i in (5, 6, 7):
                tail_finish(i)

    nc.compile()
    return nc


def _get_nc():
    if "nc" not in _CACHE:
        _CACHE["nc"] = _build()
    return _CACHE["nc"]


def _prep_inputs(x, mask, freqs, Wq, bq, Wk, bk, Wv, bv, Wo, bo):
    f = np.asarray(freqs, np.float32)[0]              # [S, HEAD_DIM]
    # reference rotates only the first rot_dim=64 channels of the FLAT
    # inner dim -> rows 0-63 of row-tile 0 on the hg=0 core; everything
    # else is identity (cos=1, sin=0).
    cos2 = np.ones((128, S), np.float32)
    sin2 = np.zeros((128, S), np.float32)
    cos2[0:HEAD_DIM] = np.cos(f.T)
    sin2[0:HEAD_DIM] = np.sin(f.T)
    ident = np.ones((128, S), np.float32)
    identz = np.zeros((128, S), np.float32)

    prt = np.zeros((128, 128), np.float16)            # P_rot^T
    i = np.arange(0, 128, 2)
    prt[i + 1, i] = -1.0                              # P_rot[2i, 2i+1] = -1
    prt[i, i + 1] = 1.0                               # P_rot[2i+1, 2i] = +1

    def lhsT_w(w):                       # [DIM, DSH] -> [128, MT, KT, 128]
        return np.ascontiguousarray(
            w.reshape(KT, 128, MT, 128).transpose(1, 2, 0, 3)
        ).astype(np.float16)

    def col(b):                                       # [DSH] -> [128, MT]
        return np.ascontiguousarray(b.reshape(MT, 128).T.astype(np.float32))

    in_maps = []
    for b in range(B):
        xT = np.ascontiguousarray(
            np.asarray(x[b], np.float32).T.reshape(KT, 128, S)
            .transpose(1, 0, 2)).astype(np.float16)
        m = np.asarray(mask[b])
        maskb = np.ascontiguousarray(
            np.where(m, 0.0, MASK_NEG).astype(np.float32).reshape(ST, 128).T)
        for hg in range(HG):
            dsl = slice(hg * DSH, (hg + 1) * DSH)
            in_maps.append({
                "xT": xT,
                "wq": lhsT_w(np.asarray(Wq, np.float32)[:, dsl]),
                "wk": lhsT_w(np.asarray(Wk, np.float32)[:, dsl]),
                "wv": np.ascontiguousarray(
                    np.asarray(Wv, np.float32)[:, dsl]
                    .reshape(KT, 128, DSH).transpose(1, 0, 2)).astype(np.float16),
                "wo": np.ascontiguousarray(
                    np.asarray(Wo, np.float32)[dsl, :]
                    .reshape(MT, 128, DIM).transpose(1, 0, 2)).astype(np.float16),
                "bq": col(np.asarray(bq, np.float32)[dsl]),
                "bk": col(np.asarray(bk, np.float32)[dsl]),
                "bv": np.asarray(bv, np.float32)[None, dsl]
                    .astype(np.float16).copy(),
                "cos2": cos2 if hg == 0 else ident,
                "sin2": sin2 if hg == 0 else identz,
                "prt": prt,
                "maskb": maskb,
            })
    return in_maps


def run(trace=False, **inputs):
    from concourse import bass_utils
    if trace:
        _install_ntff_hook()
    nc = _get_nc()
    in_maps = _prep_inputs(**inputs)
    res = bass_utils.run_bass_kernel_spmd(
        nc, in_maps, core_ids=list(range(NCORES)), trace=trace)
    mask = np.asarray(inputs["mask"])
    bo = np.asarray(inputs["bo"], np.float32)
    out = np.empty((B, S, DIM), np.float32)
    for b in range(B):
        p = (res.results[2 * b]["out"].astype(np.float32)
             + res.results[2 * b + 1]["out"].astype(np.float32) + bo)
        out[b] = np.where(mask[b][:, None], p, 0.0)
    return out, res


def kernel(**inputs):
    out, _ = run(trace=False, **inputs)
    return out


def _install_ntff_hook():
    """Register the axon NTFF profiling hook missing from the antenv stub."""
    import sys, types
    try:
        import antenv.axon_hooks  # noqa: F401
        return
    except ImportError:
        pass
    from trn_agent_boot.trn_boot import _ntff_profile_via_ctypes
    hook = _ntff_profile_via_ctypes('/opt/axon/libaxon_pjrt.so')
    mod = types.ModuleType('antenv.axon_hooks')
    mod.get_axon_ntff_profile_hook = lambda: hook
    mod.set_axon_ntff_profile_hook = lambda h: None
    sys.modules['antenv.axon_hooks'] = mod



# revision 57
# speedup vs baseline: 1.1157x; 1.0565x over previous
"""Trainium2 Bass kernel for nn_Attention (B=4, S=1024, DIM=1024, H=16, Dh=64).

Sharding: 8 cores = 4 batches x 2 head-groups (8 heads / 512 inner channels
each).  Each core computes q/k/v projections for its head shard, RoPE,
attention, and a partial output projection (its rows of Wo); the host sums
the two head-group partials per batch, adds the output bias and applies the
query mask (the cheap elementwise epilogue of the unshard step).

Device dataflow (per core), matmul operands in fp16 (fp32 PSUM accumulate):
  inputs stream in per-kt chunks on three HW DMA queues; the first 7
  projection chains (K/Q row-tile 0 + V st 0-2) interleave per-kt so the PE
  starts ~1us in and never waits on HBM.
  Q^T,K^T = W^T @ x^T  (bias added on the PSUM->SBUF pass); RoPE on the
  first 64 flat channels (host sends cos=1/sin=0 elsewhere).
  Phase 2 runs q-half (c2) OUTER, head-pair row-tile (mt) inner:
    scores^T[k,q] = K_h @ Q_h^T      (row-tiled pair)
    P^T = exp(scores^T/8 + maskb[k]) (one ACT op per (mt,kt) over 2h x 512q)
    attn^T += [V^T | 1] @ P^T        (M=65: softmax rowsum lands at output
                                      row 64, replacing separate M=1
                                      rowsum matmuls)
  as one flat 64-group software pipeline (PV lags the exp by one group,
  crossing mt/c2 boundaries) with the mt1-3 K/Q projection matmuls
  zippered into the q-half-0 groups and the q-half-0 output-projection
  chains zippered into the q-half-1 groups, so the PE stays dense while
  ACT streams exps.  NOTE: zipper chains must be fully EMITTED before the
  lookahead scores emission that reads their output (emission order
  defines Tile's dependencies).
  Normalization: approx-reciprocal of rowsums staged through partition 0
  (the custom-DVE recip only works on base-partition-0 SBUF operands --
  HW-probed); DRAM-bounce partition broadcast + gpsimd multiply wherever
  slack allows, PE ones-matmul broadcast + DVE multiply for the last
  tiles (SWDGE bounce latency would sit on the tail).
  Tail: the 8 q-half-1 output chains are software-pipelined over 3 PSUM
  rings -- all mt0-2 matmuls issue immediately; only each chain's mt3
  matmul gates on the final normalize.
  out[q,:] = attn_norm^T.T @ Wo_shard, written as fp16; host adds bo and
  masks.
"""

import numpy as np

B, S, DIM, HEADS, HEAD_DIM = 4, 1024, 1024, 16, 64
INNER = HEADS * HEAD_DIM
HG = 2                      # head groups (tensor-parallel shards)
DSH = INNER // HG           # 512 inner channels per core
HSH = HEADS // HG           # 8 heads per core
NCORES = B * HG
KT = DIM // 128             # 8 contraction tiles
MT = DSH // 128             # 4 row tiles (head pairs)
ST = S // 128               # 8 seq tiles
MASK_NEG = -80.0

_CACHE = {}


def _build():
    import concourse.tile as tile
    from concourse import bacc, mybir

    f32 = mybir.dt.float32
    f16 = mybir.dt.float16
    AF = mybir.ActivationFunctionType
    OP = mybir.AluOpType

    nc = bacc.Bacc("TRN2", target_bir_lowering=False, debug=False)

    xT_d = nc.dram_tensor("xT", [128, KT, S], f16, kind="ExternalInput")
    wq_d = nc.dram_tensor("wq", [128, MT, KT, 128], f16, kind="ExternalInput")
    wk_d = nc.dram_tensor("wk", [128, MT, KT, 128], f16, kind="ExternalInput")
    wv_d = nc.dram_tensor("wv", [128, KT, DSH], f16, kind="ExternalInput")
    wo_d = nc.dram_tensor("wo", [128, MT, DIM], f16, kind="ExternalInput")
    bq_d = nc.dram_tensor("bq", [128, MT], f32, kind="ExternalInput")
    bk_d = nc.dram_tensor("bk", [128, MT], f32, kind="ExternalInput")
    bv_d = nc.dram_tensor("bv", [1, DSH], f16, kind="ExternalInput")
    cos_d = nc.dram_tensor("cos2", [128, S], f32, kind="ExternalInput")
    sin_d = nc.dram_tensor("sin2", [128, S], f32, kind="ExternalInput")
    prt_d = nc.dram_tensor("prt", [128, 128], f16, kind="ExternalInput")
    maskb_d = nc.dram_tensor("maskb", [128, ST], f32, kind="ExternalInput")
    out_d = nc.dram_tensor("out", [S, DIM], f16, kind="ExternalOutput")
    recd = nc.dram_tensor("recd", [97, MT, 512], f16)

    with tile.TileContext(nc) as tc, \
         tc.tile_pool(name="sb", bufs=1) as sb, \
         tc.tile_pool(name="psp", bufs=1, space="PSUM") as psp:

        # ---- persistent SBUF ------------------------------------------
        xT = sb.tile([128, KT, S], f16)
        wq = sb.tile([128, MT, KT, 128], f16)
        wk = sb.tile([128, MT, KT, 128], f16)
        wv = sb.tile([128, KT, DSH], f16)
        wo = sb.tile([128, MT, DIM], f16)
        bq = sb.tile([128, MT], f32)
        bk = sb.tile([128, MT], f32)
        bvb = sb.tile([128, HSH, HEAD_DIM], f16)
        cos2 = sb.tile([128, S], f32)
        sin2 = sb.tile([128, S], f32)
        prt = sb.tile([128, 128], f16)
        maskb = sb.tile([128, ST], f32)
        qT = sb.tile([128, MT, S], f16)
        kT = sb.tile([128, MT, S], f16)
        # V with a ones column appended per head: the PV matmul (M=65)
        # then produces the softmax rowsum at output row 64 for free,
        # replacing the 128 M=1 rowsum matmuls (~20us of PE streaming).
        vv = sb.tile([128, ST, HSH, HEAD_DIM + 1], f16)
        attU = sb.tile([128, MT, S], f16)
        # rowsum staging at partition 0: reciprocal_approx_fast only works
        # with base-partition-0 SBUF operands (HW-probed; base 64 or PSUM
        # input reads garbage)
        rss2 = sb.tile([1, 2, 512], f32)
        recf2 = sb.tile([1, 2, 512], f32)
        recq = sb.tile([97, MT, 512], f16)
        ones64 = sb.tile([97, HEAD_DIM], f16)

        # ---- input DMAs: per-kt chunks on three HW DMA queues ---------
        # sync+vector: xT halves; gpsimd: wv; scalar: wk/wq (mt0 first),
        # smalls, rest, wo.
        nc.gpsimd.dma_start(
            out=bvb[:].rearrange("p h d -> p (h d)"),
            in_=bv_d.ap()[0:1, :].partition_broadcast(128))
        for kt in range(KT):
            nc.sync.dma_start(out=xT[:, kt, 0:512],
                              in_=xT_d.ap()[:, kt, 0:512])
            nc.sync.dma_start(out=xT[:, kt, 512:1024],
                              in_=xT_d.ap()[:, kt, 512:1024])
            nc.gpsimd.dma_start(out=wv[:, kt], in_=wv_d.ap()[:, kt])
            nc.scalar.dma_start(out=wk[:, 0, kt], in_=wk_d.ap()[:, 0, kt])
            nc.scalar.dma_start(out=wq[:, 0, kt], in_=wq_d.ap()[:, 0, kt])
            if kt == 1:
                for t, d in [(bq, bq_d), (bk, bk_d), (maskb, maskb_d)]:
                    nc.scalar.dma_start(out=t[:], in_=d.ap())
        nc.scalar.dma_start(out=prt[:], in_=prt_d.ap())
        nc.scalar.dma_start(out=cos2[:], in_=cos_d.ap())
        nc.scalar.dma_start(out=sin2[:], in_=sin_d.ap())
        for mt in range(1, MT):
            nc.scalar.dma_start(out=wk[:, mt], in_=wk_d.ap()[:, mt])
            nc.scalar.dma_start(out=wq[:, mt], in_=wq_d.ap()[:, mt])
        for mt in range(MT):
            nc.scalar.dma_start(out=wo[:, mt], in_=wo_d.ap()[:, mt])

        ones_f = sb.tile([128, S], f32)
        nc.vector.memset(ones_f[:], 1.0)
        nc.vector.memset(
            vv[:, :, :, HEAD_DIM:HEAD_DIM + 1]
            .rearrange("p s h one -> p (s h one)"), 1.0)
        # only rows {0,32,64,96} of recq get real rowsums, but the bounce
        # DMA ships all 97 rows — initialize so the unused rows are defined
        nc.vector.memset(recq[:], 0.0)
        nc.vector.tensor_copy(ones64[:], ones_f[0:97, 0:HEAD_DIM])
        # tiny dummy exp: pulls the ~2.7us ACT table load into the DMA fill
        # and keeps the tile scheduler's model of the first real exps tight
        warm = sb.tile([1, 8], f32)
        with tc.high_priority():
            nc.scalar.activation(warm[:], ones_f[0:1, 0:8], AF.Exp)

        def rope_apply(dst, b, c2, ps, sbpool):
            # row-tile 0 only: RoPE on the first 64 flat channels (rows
            # 64-127 and the hg=1 core get identity via cos=1/sin=0).
            sl = slice(c2 * 512, (c2 + 1) * 512)
            sinp = sbpool.tile([128, 512], f16, tag="sinp", name="sinp")
            nc.vector.scalar_tensor_tensor(
                sinp[:], ps[:], b[:, 0:1], sin2[:, sl],
                op0=OP.add, op1=OP.mult)
            cosp = sbpool.tile([128, 512], f32, tag="cosp", name="cosp")
            nc.vector.scalar_tensor_tensor(
                cosp[:], ps[:], b[:, 0:1], cos2[:, sl],
                op0=OP.add, op1=OP.mult)
            pp = psp.tile([128, 512], f32, tag="ps", name="pp")
            nc.tensor.matmul(out=pp[:], lhsT=prt[:], rhs=sinp[:],
                             start=True, stop=True)
            nc.vector.tensor_tensor(dst[:, 0, sl], cosp[:], pp[:], op=OP.add)

        # ---- prologue: mt0 K/Q chains + V, interleaved per kt ---------
        # so compute paces with the arriving per-kt DMA chunks.
        with tc.tile_pool(name="pkq", bufs=4, space="PSUM") as pkq, \
             tc.tile_pool(name="pvv", bufs=3, space="PSUM") as pvv, \
             tc.tile_pool(name="psb", bufs=2) as psb:

            def v_spill(st, ps):
                # bias add fused into the PSUM->SBUF spill (writes only the
                # 64 real V channels; col 64 stays the memset ones)
                nc.vector.tensor_tensor(
                    vv[:, st, :, 0:HEAD_DIM],
                    ps[:].rearrange("p (h d) -> p h d", h=HSH),
                    bvb[:], op=OP.add)

            kq = []
            for dst, w, b in ((kT, wk, bk), (qT, wq, bq)):
                for c2 in range(2):
                    kq.append((dst, w, b, c2,
                               pkq.tile([128, 512], f32, tag="kq",
                                        name="kq")))
            vps = [pvv.tile([128, DSH], f32, tag="vps", name="vps")
                   for _ in range(3)]
            for kt in range(KT):
                for dst, w, b, c2, ps in kq:
                    nc.tensor.matmul(
                        out=ps[:], lhsT=w[:, 0, kt, :],
                        rhs=xT[:, kt, c2 * 512:(c2 + 1) * 512],
                        start=(kt == 0), stop=(kt == KT - 1))
                for st in range(3):
                    nc.tensor.matmul(
                        out=vps[st][:],
                        lhsT=xT[:, kt, st * 128:(st + 1) * 128],
                        rhs=wv[:, kt, :],
                        start=(kt == 0), stop=(kt == KT - 1))
            for st in range(3):
                v_spill(st, vps[st])

            # interleave the rope chains (DVE-heavy) and the K-mt1 chains
            # with the remaining V chains so neither engine idles in the
            # DMA-paced prologue tail
            def v_chain(st):
                ps = pvv.tile([128, DSH], f32, tag="vps", name="vps")
                for kt in range(KT):
                    nc.tensor.matmul(
                        out=ps[:], lhsT=xT[:, kt, st * 128:(st + 1) * 128],
                        rhs=wv[:, kt, :],
                        start=(kt == 0), stop=(kt == KT - 1))
                v_spill(st, ps)

            def k1_chain(half):
                ps = pvv.tile([128, DSH], f32, tag="vps", name="kx")
                sl = slice(half * 512, (half + 1) * 512)
                for kt in range(KT):
                    nc.tensor.matmul(
                        out=ps[:], lhsT=wk[:, 1, kt, :], rhs=xT[:, kt, sl],
                        start=(kt == 0), stop=(kt == KT - 1))
                nc.vector.tensor_scalar(
                    kT[:, 1, sl], ps[:], bk[:, 1:2], None, op0=OP.add)

            for st in range(3, ST):
                if st - 3 < len(kq):
                    dst, w, b, c2, ps = kq[st - 3]
                    rope_apply(dst, b, c2, ps, psb)
                v_chain(st)
                if st == 3:
                    k1_chain(0)
                if st == 4:
                    k1_chain(1)

        # ---- zipper chains (run inside phase-2 groups) ----------------
        def chain_proj(dst, w, b, mt, half):
            sl = slice(half * 512, (half + 1) * 512)
            ps = psp.tile([128, 512], f32, tag="ps", name="ps")
            for kt in range(KT):
                nc.tensor.matmul(
                    out=ps[:], lhsT=w[:, mt, kt, :], rhs=xT[:, kt, sl],
                    start=(kt == 0), stop=(kt == KT - 1))
                if kt == KT - 1:
                    nc.vector.tensor_scalar(
                        dst[:, mt, sl], ps[:], b[:, mt:mt + 1],
                        None, op0=OP.add)
                yield

        def chain_out(qt, n2, pool, tag, obpool, copy_eng, dma_eng=None):
            nsl = slice(n2 * 512, (n2 + 1) * 512)
            ps = pool.tile([128, 512], f32, tag=tag, name="ps3")
            for mt in range(MT):
                nc.tensor.matmul(
                    out=ps[:], lhsT=attU[:, mt, qt * 128:(qt + 1) * 128],
                    rhs=wo[:, mt, nsl],
                    start=(mt == 0), stop=(mt == MT - 1))
                if mt == MT - 1:
                    ob = obpool.tile([128, 512], f16, tag="ob", name="ob")
                    if copy_eng == "act":
                        nc.scalar.activation(ob[:], ps[:], AF.Copy)
                    else:
                        nc.vector.tensor_copy(ob[:], ps[:])
                    eng = dma_eng
                    if eng is None:
                        eng = nc.sync if (qt + n2) % 2 == 0 else nc.scalar
                    eng.dma_start(
                        out=out_d.ap()[qt * 128:(qt + 1) * 128, nsl],
                        in_=ob[:])
                yield

        # ---- phase 2+3: attention with zippered projections/output ----
        with tc.tile_pool(name="psc", bufs=2, space="PSUM") as psc, \
             tc.tile_pool(name="pat0", bufs=2, space="PSUM") as pat0, \
             tc.tile_pool(name="pat1", bufs=1, space="PSUM") as pat1, \
             tc.tile_pool(name="p2sb", bufs=3) as p2sb, \
             tc.tile_pool(name="p2r", bufs=2) as p2r:

            def emit_scores(mt, kt, c2):
                sch = psc.tile([128, S], f32, tag="sch", name="sch")
                qsl = slice(c2 * 512, (c2 + 1) * 512)
                for hh in range(2):
                    ph = hh * 64
                    nc.tensor.matmul(
                        out=sch[:, hh * 512:hh * 512 + 512],
                        lhsT=kT[ph:ph + 64, mt, kt * 128:(kt + 1) * 128],
                        rhs=qT[ph:ph + 64, mt, qsl],
                        start=True, stop=True, tile_position=(ph, 0))
                return sch

            def normalize_bounce(mt, c2):
                # rows 32*(2hh+c2) of recq hold 1/rowsum for head (2mt+hh),
                # q-half c2.  DRAM bounce partition-broadcasts each row; the
                # multiply runs on the otherwise-idle GpSimd engine.
                nc.gpsimd.dma_start(out=recd.ap()[:, mt, :],
                                    in_=recq[:, mt, :])
                qsl = slice(c2 * 512, (c2 + 1) * 512)
                for hh in range(2):
                    ph, r = hh * 64, 32 * (hh * 2 + c2)
                    rb = p2r.tile([128, 512], f32, tag="rb", name="rb")
                    nc.gpsimd.dma_start(
                        out=rb[ph:ph + 64],
                        in_=recd.ap()[r:r + 1, mt, :].partition_broadcast(64))
                    nc.gpsimd.tensor_tensor(
                        attU[ph:ph + 64, mt, qsl], attU[ph:ph + 64, mt, qsl],
                        rb[ph:ph + 64], op=OP.mult)

            def normalize_pe(mt, c2):
                # PE ones-matmul broadcast (for the final tile where the
                # DMA bounce latency would sit exposed)
                qsl = slice(c2 * 512, (c2 + 1) * 512)
                rbps = psp.tile([128, 512], f32, tag="ps", name="rbps")
                for hh in range(2):
                    ph, r = hh * 64, 32 * (hh * 2 + c2)
                    nc.tensor.matmul(
                        out=rbps[ph:ph + 64, :], lhsT=ones64[r:r + 1, :],
                        rhs=recq[r:r + 1, mt, :],
                        start=True, stop=True, tile_position=(r, ph))
                for hh in range(2):
                    ph = hh * 64
                    nc.vector.tensor_tensor(
                        attU[ph:ph + 64, mt, qsl], attU[ph:ph + 64, mt, qsl],
                        rbps[ph:ph + 64, :], op=OP.mult)

            def gen_c2_0():
                # order is deadline-driven: each chain must be fully
                # EMITTED before the lookahead scores emission that reads
                # it (program order defines Tile's deps); K-mt1 chains ran
                # in the prologue
                yield from chain_proj(qT, wq, bq, 1, 0)
                yield from chain_proj(qT, wq, bq, 2, 0)
                yield from chain_proj(kT, wk, bk, 2, 0)
                yield from chain_proj(kT, wk, bk, 2, 1)
                yield from chain_proj(qT, wq, bq, 3, 0)
                yield from chain_proj(kT, wk, bk, 3, 0)
                yield from chain_proj(kT, wk, bk, 3, 1)
                yield from chain_proj(qT, wq, bq, 1, 1)
                while True:
                    yield

            def gen_c2_1():
                yield from chain_proj(qT, wq, bq, 2, 1)
                yield from chain_proj(qT, wq, bq, 3, 1)
                for qt in range(4):
                    for n2 in range(2):
                        yield from chain_out(qt, n2, psp, "ps", p2sb, "dve")
                while True:
                    yield

            # ---- flat 64-group software pipeline --------------------------
            # group g = (c2, mt, kt); exp/scores run one group AHEAD of the
            # PV/rowsum consumers, crossing mt and c2 boundaries, so neither
            # the PE nor ACT ever drains at a boundary.
            ZIPN = {0: (3, 2, 2, 1), 1: (1, 2, 2, 2)}
            groups = [(c2, mt, kt) for c2 in range(2) for mt in range(MT)
                      for kt in range(ST)]
            gens = {0: gen_c2_0(), 1: gen_c2_1()}
            NG = len(groups)
            pts, ats = {}, {}
            sch = {0: emit_scores(0, 0, 0)}
            for g in range(NG + 1):
                if g < NG:
                    c2, mt, kt = groups[g]
                    pts[g] = p2sb.tile([128, S], f16, tag="pt", name="pt")
                    nc.scalar.activation(
                        pts[g][:], sch.pop(g)[:], AF.Exp,
                        bias=maskb[:, kt:kt + 1], scale=0.125)
                    if g + 1 < NG:
                        nc2, nmt, nkt = groups[g + 1]
                        sch[g + 1] = emit_scores(nmt, nkt, nc2)
                    for _ in range(ZIPN[c2][mt]):
                        next(gens[c2])
                if g >= 1:
                    c2, mt, kt = groups[g - 1]
                    first, last = (kt == 0), (kt == ST - 1)
                    if first:
                        ats[mt, c2] = (
                            pat0.tile([128, 512], f32, tag="at0", name="at0"),
                            pat1.tile([128, 512], f32, tag="at1", name="at1"))
                    atx = ats[mt, c2]
                    pt = pts.pop(g - 1)
                    for hh in range(2):  # PV pair, rowsum row merged (M=65)
                        nc.tensor.matmul(
                            out=atx[hh][0:65, :],
                            lhsT=vv[:, kt, mt * 2 + hh, :],
                            rhs=pt[:, hh * 512:hh * 512 + 512],
                            start=first, stop=last)
                    if last:
                        # epilogue.  recip runs via partition 0 (the
                        # custom-DVE recip is base-0-only, SBUF-only).
                        # Mid-loop the at1 readers go first (its ring is
                        # single-buffered; the next group's PV waits on
                        # them); in the final epilogue the recip chain
                        # goes first -- it gates the tail's normalize.
                        qsl = slice(c2 * 512, (c2 + 1) * 512)
                        final = (c2 == 1 and mt == MT - 1)
                        nc.vector.tensor_copy(
                            rss2[0:1, 1], atx[1][64:65, :])
                        if not final:
                            nc.vector.tensor_copy(
                                attU[64:128, mt, qsl], atx[1][0:64, :])
                        nc.vector.tensor_copy(
                            rss2[0:1, 0], atx[0][64:65, :])
                        nc.vector.reciprocal_approx_fast(
                            recf2[0:1, :, :], rss2[0:1, :, :])
                        for hh in range(2):
                            r = 32 * (hh * 2 + c2)
                            nc.vector.tensor_copy(
                                recq[r:r + 1, mt, :], recf2[0:1, hh])
                        if final:
                            nc.vector.tensor_copy(
                                attU[64:128, mt, qsl], atx[1][0:64, :])
                        nc.vector.tensor_copy(
                            attU[0:64, mt, qsl], atx[0][0:64, :])
                        # bounce only for c2=0 (consumers ~30 groups out);
                        # a c2=1 bounce writes attU right before the
                        # zippered chain_outs read the SAME tile and the
                        # dep tracker stalls the PE ~10us on the SWDGE
                        # roundtrip (measured)
                        if c2 == 0 and mt < MT - 1:
                            normalize_bounce(mt, c2)
                        else:
                            normalize_pe(mt, c2)

            # ---- tail: output projection for q-half 1 -----------------
            # software-pipelined over 3 PSUM rings: every chain's mt0-2
            # MMs are ready the moment the loop ends (attU c2=1 mt0-2
            # normalized much earlier); only the mt3 MM gates on the
            # final normalize.  Emitting all ready MMs first keeps the
            # in-order PE queue busy while the last epilogue completes.
            pools = [(psp, "ps"), (pat0, "at0"), (pat1, "at1")]
            dma_engs = [nc.sync, nc.scalar, nc.gpsimd]
            chains = [(q, n) for q in range(4, 8) for n in range(2)]
            pend = {}

            def tail_finish(i):
                qt, n2, ps = pend.pop(i)
                nsl = slice(n2 * 512, (n2 + 1) * 512)
                nc.tensor.matmul(
                    out=ps[:], lhsT=attU[:, 3, qt * 128:(qt + 1) * 128],
                    rhs=wo[:, 3, nsl], start=False, stop=True)
                ob = p2sb.tile([128, 512], f16, tag="ob", name="ob")
                if i % 2 == 0:
                    nc.scalar.activation(ob[:], ps[:], AF.Copy)
                else:
                    nc.vector.tensor_copy(ob[:], ps[:])
                dma_engs[i % 3].dma_start(
                    out=out_d.ap()[qt * 128:(qt + 1) * 128, nsl], in_=ob[:])

            for i, (qt, n2) in enumerate(chains):
                if i >= 3:
                    tail_finish(i - 3)
                pool, tag = pools[i % 3]
                ps = pool.tile([128, 512], f32, tag=tag, name="ps3")
                nsl = slice(n2 * 512, (n2 + 1) * 512)
                for mt in range(3):
                    nc.tensor.matmul(
                        out=ps[:],
                        lhsT=attU[:, mt, qt * 128:(qt + 1) * 128],
                        rhs=wo[:, mt, nsl], start=(mt == 0), stop=False)
                pend[i] = (qt, n2, ps)
            for i in (5, 6, 7):
                tail_finish(i)

    nc.compile()
    return nc


def _get_nc():
    if "nc" not in _CACHE:
        _CACHE["nc"] = _build()
    return _CACHE["nc"]


def _prep_inputs(x, mask, freqs, Wq, bq, Wk, bk, Wv, bv, Wo, bo):
    f = np.asarray(freqs, np.float32)[0]              # [S, HEAD_DIM]
    # reference rotates only the first rot_dim=64 channels of the FLAT
    # inner dim -> rows 0-63 of row-tile 0 on the hg=0 core; everything
    # else is identity (cos=1, sin=0).
    cos2 = np.ones((128, S), np.float32)
    sin2 = np.zeros((128, S), np.float32)
    cos2[0:HEAD_DIM] = np.cos(f.T)
    sin2[0:HEAD_DIM] = np.sin(f.T)
    ident = np.ones((128, S), np.float32)
    identz = np.zeros((128, S), np.float32)

    prt = np.zeros((128, 128), np.float16)            # P_rot^T
    i = np.arange(0, 128, 2)
    prt[i + 1, i] = -1.0                              # P_rot[2i, 2i+1] = -1
    prt[i, i + 1] = 1.0                               # P_rot[2i+1, 2i] = +1

    def lhsT_w(w):                       # [DIM, DSH] -> [128, MT, KT, 128]
        return np.ascontiguousarray(
            w.reshape(KT, 128, MT, 128).transpose(1, 2, 0, 3)
        ).astype(np.float16)

    def col(b):                                       # [DSH] -> [128, MT]
        return np.ascontiguousarray(b.reshape(MT, 128).T.astype(np.float32))

    in_maps = []
    for b in range(B):
        xT = np.ascontiguousarray(
            np.asarray(x[b], np.float32).T.reshape(KT, 128, S)
            .transpose(1, 0, 2)).astype(np.float16)
        m = np.asarray(mask[b])
        maskb = np.ascontiguousarray(
            np.where(m, 0.0, MASK_NEG).astype(np.float32).reshape(ST, 128).T)
        for hg in range(HG):
            dsl = slice(hg * DSH, (hg + 1) * DSH)
            in_maps.append({
                "xT": xT,
                "wq": lhsT_w(np.asarray(Wq, np.float32)[:, dsl]),
                "wk": lhsT_w(np.asarray(Wk, np.float32)[:, dsl]),
                "wv": np.ascontiguousarray(
                    np.asarray(Wv, np.float32)[:, dsl]
                    .reshape(KT, 128, DSH).transpose(1, 0, 2)).astype(np.float16),
                "wo": np.ascontiguousarray(
                    np.asarray(Wo, np.float32)[dsl, :]
                    .reshape(MT, 128, DIM).transpose(1, 0, 2)).astype(np.float16),
                "bq": col(np.asarray(bq, np.float32)[dsl]),
                "bk": col(np.asarray(bk, np.float32)[dsl]),
                "bv": np.asarray(bv, np.float32)[None, dsl]
                    .astype(np.float16).copy(),
                "cos2": cos2 if hg == 0 else ident,
                "sin2": sin2 if hg == 0 else identz,
                "prt": prt,
                "maskb": maskb,
            })
    return in_maps


def run(trace=False, **inputs):
    from concourse import bass_utils
    if trace:
        _install_ntff_hook()
    nc = _get_nc()
    in_maps = _prep_inputs(**inputs)
    res = bass_utils.run_bass_kernel_spmd(
        nc, in_maps, core_ids=list(range(NCORES)), trace=trace)
    mask = np.asarray(inputs["mask"])
    bo = np.asarray(inputs["bo"], np.float32)
    out = np.empty((B, S, DIM), np.float32)
    for b in range(B):
        p = (res.results[2 * b]["out"].astype(np.float32)
             + res.results[2 * b + 1]["out"].astype(np.float32) + bo)
        out[b] = np.where(mask[b][:, None], p, 0.0)
    return out, res


def kernel(**inputs):
    out, _ = run(trace=False, **inputs)
    return out


def _install_ntff_hook():
    """Register the axon NTFF profiling hook missing from the antenv stub."""
    import sys, types
    try:
        import antenv.axon_hooks  # noqa: F401
        return
    except ImportError:
        pass
    from trn_agent_boot.trn_boot import _ntff_profile_via_ctypes
    hook = _ntff_profile_via_ctypes('/opt/axon/libaxon_pjrt.so')
    mod = types.ModuleType('antenv.axon_hooks')
    mod.get_axon_ntff_profile_hook = lambda: hook
    mod.set_axon_ntff_profile_hook = lambda h: None
    sys.modules['antenv.axon_hooks'] = mod

